# revision 5
# baseline (speedup 1.0000x reference)
"""Trainium2 Bass kernel for nn_ART_block (gnn_message_passing).
Data-parallel over B=8 graphs across 8 NeuronCores. Self-contained.
"""
import numpy as np
import ml_dtypes
import concourse.bass as bass
import concourse.mybir as mybir
import concourse.tile as tile
from concourse import bacc
from concourse.bass_utils import run_bass_kernel_spmd

f32 = mybir.dt.float32
bf16 = mybir.dt.bfloat16
i16 = mybir.dt.int16
AL = mybir.AluOpType
AF = mybir.ActivationFunctionType

B, N, E, D = 8, 256, 65536, 512
NT = E // 128          # 512 tiles of 128 edges
NB = NT // 4           # 128 batches of 4 tiles
EPS = 1e-5

_CACHED_NC = None
LAST_RESULTS = None


def _build_nc():
    nc = bacc.Bacc("TRN2", target_bir_lowering=False, debug=False, num_devices=8)
    # ---- inputs (per-core shard) ----
    uti = nc.dram_tensor("uti", [NT, 128, 512], f32, kind="ExternalInput")
    obj2 = nc.dram_tensor("obj2", [2, 128, 512], f32, kind="ExternalInput")
    objT4 = nc.dram_tensor("objT4", [4, 128, 256], f32, kind="ExternalInput")
    wswT4 = nc.dram_tensor("wswT4", [4, 128, 512], f32, kind="ExternalInput")
    wowT4 = nc.dram_tensor("wowT4", [4, 128, 512], f32, kind="ExternalInput")
    convwT4 = nc.dram_tensor("convwT4", [4, 128, 512], f32, kind="ExternalInput")
    t1wT4 = nc.dram_tensor("t1wT4", [4, 128, 1024], f32, kind="ExternalInput")
    t2wT8 = nc.dram_tensor("t2wT8", [8, 128, 512], f32, kind="ExternalInput")
    wsb4 = nc.dram_tensor("wsb4", [128, 4], f32, kind="ExternalInput")
    wob4 = nc.dram_tensor("wob4", [128, 4], f32, kind="ExternalInput")
    wwT4 = nc.dram_tensor("wwT4", [128, 4], f32, kind="ExternalInput")
    t1b8 = nc.dram_tensor("t1b8", [128, 8], f32, kind="ExternalInput")
    convb_b = nc.dram_tensor("convb_b", [128, 512], f32, kind="ExternalInput")
    t2b_b = nc.dram_tensor("t2b_b", [128, 512], f32, kind="ExternalInput")
    ln1g_b = nc.dram_tensor("ln1g_b", [128, 512], f32, kind="ExternalInput")
    ln1b_b = nc.dram_tensor("ln1b_b", [128, 512], f32, kind="ExternalInput")
    ln2g_b = nc.dram_tensor("ln2g_b", [128, 512], f32, kind="ExternalInput")
    ln2b_b = nc.dram_tensor("ln2b_b", [128, 512], f32, kind="ExternalInput")
    wb_rep = nc.dram_tensor("wb_rep", [128, 1], f32, kind="ExternalInput")
    icol = nc.dram_tensor("icol", [128, NT], f32, kind="ExternalInput")
    jcol = nc.dram_tensor("jcol", [128, NT], f32, kind="ExternalInput")
    gidx = nc.dram_tensor("gidx", [128, NB * 64], i16, kind="ExternalInput")
    # ---- outputs ----
    refined_o = nc.dram_tensor("refined_o", [2, 128, 512], f32, kind="ExternalOutput")
    att_o = nc.dram_tensor("att_o", [2, 128, 256], f32, kind="ExternalOutput")
    ctx_o = nc.dram_tensor("ctx_o", [2, 128, 512], f32, kind="ExternalOutput")

    # ---- inline consts ----
    iota_c = nc.inline_tensor(np.tile(np.arange(256), (128, 1)).astype(ml_dtypes.bfloat16), name="iota_c")
    ones_c = nc.inline_tensor(np.ones((128, 1), dtype=ml_dtypes.bfloat16), name="ones_c")
    identb_c = nc.inline_tensor(np.eye(128).astype(ml_dtypes.bfloat16), name="identb_c")
    identf_c = nc.inline_tensor(np.eye(128, dtype=np.float32), name="identf_c")
    eye_np = np.zeros((2, 128, 256), np.float32)
    for h in range(2):
        for p in range(128):
            eye_np[h, p, h * 128 + p] = 10000.0
    eye_c = nc.inline_tensor(eye_np, name="eye_c")

    with tile.TileContext(nc) as tc:
        with (
            tc.tile_pool(name="pers", bufs=1) as P,           # persistent SBUF
            tc.tile_pool(name="loop", bufs=3) as L,           # streaming tiles
            tc.tile_pool(name="psA", bufs=1, space="PSUM") as PSA,   # att banks
            tc.tile_pool(name="psF", bufs=2, space="PSUM") as PSF,   # atten_f
            tc.tile_pool(name="psW", bufs=2, space="PSUM") as PSW,   # prologue/epilogue work
        ):
            # ---------- preload params ----------
            def load(pool, dram, shape, dtype=f32):
                t = pool.tile(shape, dtype, name=dram.name + "_sb")
                nc.sync.dma_start(t[:], dram.ap())
                return t

            iota_t = load(P, iota_c, [128, 256], bf16)
            ones_t = load(P, ones_c, [128, 1], bf16)
            identb = load(P, identb_c, [128, 128], bf16)
            eye_t = P.tile([128, 2, 256], f32)
            nc.sync.dma_start(eye_t[:], eye_c.ap().rearrange("h p b -> p h b"))
            objs = P.tile([128, 2, 512], f32)
            nc.sync.dma_start(objs[:], obj2.ap().rearrange("h p e -> p h e"))
            objT = P.tile([128, 4, 256], f32)
            nc.sync.dma_start(objT[:], objT4.ap().rearrange("k p a -> p k a"))
            wswT = P.tile([128, 4, 512], f32)
            nc.sync.dma_start(wswT[:], wswT4.ap().rearrange("k p a -> p k a"))
            wowT = P.tile([128, 4, 512], f32)
            nc.sync.dma_start(wowT[:], wowT4.ap().rearrange("k p a -> p k a"))



            wsb = load(P, wsb4, [128, 4])
            wob = load(P, wob4, [128, 4])
            wwT = load(P, wwT4, [128, 4])
            t1b = load(P, t1b8, [128, 8])
            convb = load(P, convb_b, [128, 512])
            t2bb = load(P, t2b_b, [128, 512])
            ln1g = load(P, ln1g_b, [128, 512])
            ln1b = load(P, ln1b_b, [128, 512])
            ln2g = load(P, ln2g_b, [128, 512])
            ln2b = load(P, ln2b_b, [128, 512])
            wbr = load(P, wb_rep, [128, 1])
            icolt = load(P, icol, [128, NT])
            jcolt = load(P, jcol, [128, NT])
            gidxt = load(P, gidx, [128, NB * 64], i16)

            # ---------- prologue: gather tables s'T, oT (interleaved bf16) ----------
            tabS = P.tile([128, 256, 4], bf16)
            tabO = P.tile([128, 256, 4], bf16)
            for c in range(4):
                mm = PSW.tile([128, 256], f32, space="PSUM", tag="w")
                for k in range(4):
                    nc.tensor.matmul(out=mm[:], lhsT=wswT[:, k, c * 128:(c + 1) * 128],
                                     rhs=objT[:, k, :], start=(k == 0), stop=(k == 3))
                nc.vector.tensor_scalar(out=tabS[:, :, c], in0=mm[:],
                                        scalar1=wsb[:, c:c + 1], scalar2=wwT[:, c:c + 1],
                                        op0=AL.add, op1=AL.mult)
                mo = PSW.tile([128, 256], f32, space="PSUM", tag="w")
                for k in range(4):
                    nc.tensor.matmul(out=mo[:], lhsT=wowT[:, k, c * 128:(c + 1) * 128],
                                     rhs=objT[:, k, :], start=(k == 0), stop=(k == 3))
                nc.vector.tensor_scalar(out=tabO[:, :, c], in0=mo[:],
                                        scalar1=wob[:, c:c + 1], scalar2=None, op0=AL.add)

            # ---------- prologue: conv path c16 = relu(LN1(obj) @ conv_w.T) ----------
            def layernorm(src_h, g_t, b_t, out16_h, tag):
                # src_h: [128,512] f32 SBUF; writes bf16 normalized out
                s1 = P.tile([128, 1], f32, tag=tag + "s1")
                nc.vector.tensor_reduce(out=s1[:], in_=src_h[:], axis=mybir.AxisListType.X, op=AL.add)
                nm = P.tile([128, 1], f32, tag=tag + "nm")
                nc.vector.tensor_scalar(out=nm[:], in0=s1[:], scalar1=-1.0 / 512, scalar2=None, op0=AL.mult)
                xm = P.tile([128, 512], f32, tag="lnxm", name="lnxm")
                nc.scalar.activation(out=xm[:], in_=src_h[:], func=AF.Identity, bias=nm[:], scale=1.0)
                ssq = P.tile([128, 1], f32, tag=tag + "sq")
                junk = P.tile([128, 512], bf16, tag="lnjk", name="lnjk")
                nc.vector.scalar_tensor_tensor(out=junk[:], in0=xm[:], scalar=1.0, in1=xm[:],
                                               op0=AL.mult, op1=AL.mult, accum_out=ssq[:])
                var = P.tile([128, 1], f32, tag=tag + "vr")
                nc.vector.tensor_scalar(out=var[:], in0=ssq[:], scalar1=1.0 / 512, scalar2=EPS,
                                        op0=AL.mult, op1=AL.add)
                sd = P.tile([128, 1], f32, tag=tag + "sd")
                nc.scalar.activation(out=sd[:], in_=var[:], func=AF.Sqrt)
                rs = P.tile([128, 1], f32, tag=tag + "rs")
                nc.vector.reciprocal(rs[:], sd[:])
                xn = P.tile([128, 512], f32, tag="lnxn", name="lnxn")
                nc.scalar.activation(out=xn[:], in_=xm[:], func=AF.Copy, scale=rs[:])
                xg = P.tile([128, 512], f32, tag="lnxg", name="lnxg")
                nc.vector.tensor_tensor(out=xg[:], in0=xn[:], in1=g_t[:], op=AL.mult)
                xf = P.tile([128, 512], f32, tag="lnxf", name="lnxf")
                nc.vector.tensor_tensor(out=xf[:], in0=xg[:], in1=b_t[:], op=AL.add)
                nc.vector.tensor_copy(out16_h[:], xf[:])
                return xf

            ln1_16 = [P.tile([128, 512], bf16, tag=f"l1_{h}", name=f"l1_{h}") for h in range(2)]
            for h in range(2):
                layernorm(objs[:, h, :], ln1g, ln1b, ln1_16[h], f"ln1{h}")
            # transpose LN1 -> lnT [4][128dp, 256b] bf16
            lnT = P.tile([128, 4, 256], bf16)
            for dk in range(4):
                tp = PSW.tile([128, 256], bf16, space="PSUM", tag="w")
                for h in range(2):
                    nc.tensor.transpose(out=tp[:, h * 128:(h + 1) * 128],
                                        in_=ln1_16[h][:, dk * 128:(dk + 1) * 128], identity=identb[:])
                nc.vector.tensor_copy(lnT[:, dk, :], tp[:])
            convwT16 = P.tile([128, 4, 512], bf16)
            nc.gpsimd.dma_start(convwT16[:], convwT4.ap().rearrange("k p a -> p k a"))
            c16 = P.tile([128, 2, 512], bf16)
            for h in range(2):
                cp = PSW.tile([128, 512], f32, space="PSUM", tag="w")
                for dk in range(4):
                    nc.tensor.matmul(out=cp[:], lhsT=lnT[:, dk, h * 128:(h + 1) * 128],
                                     rhs=convwT16[:, dk, :], start=(dk == 0), stop=(dk == 3))
                cb = P.tile([128, 512], f32, tag=f"cb{h}")
                nc.vector.tensor_tensor(out=cb[:], in0=cp[:], in1=convb[:], op=AL.add)
                nc.vector.tensor_scalar(out=c16[:, h, :], in0=cb[:], scalar1=0.0, scalar2=None, op0=AL.max)

            # ---------- att PSUM init (start=True zero matmul) ----------
            att_ps = [PSA.tile([128, 256], f32, space="PSUM", tag=f"att{h}", name=f"att{h}") for h in range(2)]
            zero16 = P.tile([128, 256], bf16)
            nc.vector.memset(zero16[:], 0.0)
            for h in range(2):
                nc.tensor.matmul(out=att_ps[h][:], lhsT=zero16[:, :128], rhs=zero16[:],
                                 start=True, stop=False)

            # ---------- streaming loop over batches ----------
            def batch_body(bi):
                g1 = L.tile([128, 512, 4], bf16, tag="g1")
                g2 = L.tile([128, 512, 4], bf16, tag="g2")
                nc.gpsimd.ap_gather(out_ap=g1[:], in_ap=tabS[:], idxs_ap=gidxt[:, bass.ds(bi * 64, 32)],
                                    channels=128, num_elems=256, d=4, num_idxs=512)
                nc.gpsimd.ap_gather(out_ap=g2[:], in_ap=tabO[:], idxs_ap=gidxt[:, bass.ds(bi * 64 + 32, 32)],
                                    channels=128, num_elems=256, d=4, num_idxs=512)
                for k in range(4):
                    t = bi * 4 + k
                    u16 = L.tile([128, 512], bf16, tag="u16")
                    nc.gpsimd.dma_start(u16[:], uti.ap()[bass.ds(t, 1)].rearrange("o p e -> (o p) e"))
                    mi = L.tile([128, 128, 4], bf16, tag="mi")
                    nc.vector.tensor_tensor(out=mi[:].rearrange("p a b -> p (a b)"),
                                            in0=g1[:, k * 128:(k + 1) * 128, :].rearrange("p a b -> p (a b)"),
                                            in1=g2[:, k * 128:(k + 1) * 128, :].rearrange("p a b -> p (a b)"),
                                            op=AL.mult)
                    pr = L.tile([128, 128, 4], bf16, tag="pr")
                    nc.vector.tensor_tensor(out=pr[:].rearrange("p a b -> p (a b)"),
                                            in0=mi[:].rearrange("p a b -> p (a b)"),
                                            in1=u16[:].rearrange("p e -> p e"), op=AL.mult)
                    fps = PSF.tile([128, 1], f32, space="PSUM", tag="fps")
                    for c in range(4):
                        nc.tensor.matmul(out=fps[:], lhsT=pr[:, :, c], rhs=ones_t[:],
                                         start=(c == 0), stop=(c == 3))
                    fb = L.tile([128, 1], f32, tag="fb")
                    nc.vector.tensor_scalar(out=fb[:], in0=fps[:], scalar1=wbr[:], scalar2=None, op0=AL.add)
                    ohi = L.tile([128, 256], bf16, tag="ohi")
                    nc.vector.tensor_scalar(out=ohi[:], in0=iota_t[:], scalar1=icolt[:, bass.ds(t, 1)],
                                            scalar2=None, op0=AL.is_equal)
                    xt = L.tile([128, 256], bf16, tag="xt")
                    nc.vector.tensor_scalar(out=xt[:], in0=iota_t[:], scalar1=jcolt[:, bass.ds(t, 1)],
                                            scalar2=fb[:], op0=AL.is_equal, op1=AL.mult)
                    for h in range(2):
                        nc.tensor.matmul(out=att_ps[h][:], lhsT=ohi[:, h * 128:(h + 1) * 128],
                                         rhs=xt[:], start=False, stop=False)

            tc.For_i_unrolled(0, NB, 1, batch_body, max_unroll=4)

            # close psum accumulation
            for h in range(2):
                nc.tensor.matmul(out=att_ps[h][:], lhsT=zero16[:, :128], rhs=zero16[:],
                                 start=False, stop=True)

            # ---------- epilogue ----------
            att16 = P.tile([128, 2, 256], bf16)
            attf = P.tile([128, 2, 256], f32)
            for h in range(2):
                am = P.tile([128, 256], f32, tag=f"am{h}")
                nc.vector.tensor_tensor(out=am[:], in0=att_ps[h][:], in1=eye_t[:, h, :], op=AL.subtract)
                mx = P.tile([128, 1], f32, tag=f"mx{h}")
                nc.vector.tensor_reduce(out=mx[:], in_=am[:], axis=mybir.AxisListType.X, op=AL.max)
                nmx = P.tile([128, 1], f32, tag=f"nmx{h}")
                nc.vector.tensor_scalar(out=nmx[:], in0=mx[:], scalar1=-1.0, scalar2=None, op0=AL.mult)
                ex = P.tile([128, 256], f32, tag=f"ex{h}")
                sm = P.tile([128, 1], f32, tag=f"sm{h}")
                nc.scalar.activation(out=ex[:], in_=am[:], func=AF.Exp, bias=nmx[:], scale=1.0,
                                     accum_out=sm[:])
                rs = P.tile([128, 1], f32, tag=f"rsm{h}")
                nc.vector.reciprocal(rs[:], sm[:])
                nc.scalar.activation(out=attf[:, h, :], in_=ex[:], func=AF.Copy, scale=rs[:])
                nc.vector.tensor_copy(att16[:, h, :], attf[:, h, :])
                nc.sync.dma_start(att_o.ap()[h], attf[:, h, :])
            # attT for ctx matmul
            attT = P.tile([128, 2, 256], bf16)  # [128b, bk, 256a]
            for bk in range(2):
                tp = PSW.tile([128, 256], bf16, space="PSUM", tag="w")
                for h in range(2):
                    nc.tensor.transpose(out=tp[:, h * 128:(h + 1) * 128],
                                        in_=att16[:, h, bk * 128:(bk + 1) * 128], identity=identb[:])
                nc.vector.tensor_copy(attT[:, bk, :], tp[:])
            outfeat = P.tile([128, 2, 512], f32)
            ctxf = P.tile([128, 2, 512], f32)
            for h in range(2):
                cxp = PSW.tile([128, 512], f32, space="PSUM", tag="w")
                for bk in range(2):
                    nc.tensor.matmul(out=cxp[:], lhsT=attT[:, bk, h * 128:(h + 1) * 128],
                                     rhs=c16[:, bk, :], start=(bk == 0), stop=(bk == 1))
                nc.vector.tensor_copy(ctxf[:, h, :], cxp[:])
                nc.sync.dma_start(ctx_o.ap()[h], ctxf[:, h, :])
                nc.vector.tensor_tensor(out=outfeat[:, h, :], in0=cxp[:], in1=objs[:, h, :], op=AL.add)
            # LN2 + MLP
            h16 = [P.tile([128, 512], bf16, tag=f"h16_{h}", name=f"h16_{h}") for h in range(2)]
            for h in range(2):
                layernorm(outfeat[:, h, :], ln2g, ln2b, h16[h], f"ln2{h}")
            hT = P.tile([128, 4, 256], bf16)
            for dk in range(4):
                tp = PSW.tile([128, 256], bf16, space="PSUM", tag="w")
                for h in range(2):
                    nc.tensor.transpose(out=tp[:, h * 128:(h + 1) * 128],
                                        in_=h16[h][:, dk * 128:(dk + 1) * 128], identity=identb[:])
                nc.vector.tensor_copy(hT[:, dk, :], tp[:])
            t1wT16 = P.tile([128, 4, 1024], bf16)
            nc.gpsimd.dma_start(t1wT16[:], t1wT4.ap().rearrange("k p a -> p k a"))
            t2wT16 = P.tile([128, 8, 512], bf16)
            nc.gpsimd.dma_start(t2wT16[:], t2wT8.ap().rearrange("k p a -> p k a"))
            h1r = P.tile([128, 8, 256], bf16)
            for fm in range(8):
                hp = PSW.tile([128, 256], f32, space="PSUM", tag="w")
                for dk in range(4):
                    nc.tensor.matmul(out=hp[:], lhsT=t1wT16[:, dk, fm * 128:(fm + 1) * 128],
                                     rhs=hT[:, dk, :], start=(dk == 0), stop=(dk == 3))
                nc.scalar.activation(out=h1r[:, fm, :], in_=hp[:], func=AF.Relu, bias=t1b[:, fm:fm + 1], scale=1.0)
            rt16 = P.tile([128, 4, 256], bf16)
            for dm in range(4):
                rp = PSW.tile([128, 256], f32, space="PSUM", tag="w")
                for fk in range(8):
                    nc.tensor.matmul(out=rp[:], lhsT=t2wT16[:, fk, dm * 128:(dm + 1) * 128],
                                     rhs=h1r[:, fk, :], start=(fk == 0), stop=(fk == 7))
                nc.vector.tensor_copy(rt16[:, dm, :], rp[:])
            for h in range(2):
                rn = PSW.tile([128, 512], bf16, space="PSUM", tag="w")
                for dm in range(4):
                    nc.tensor.transpose(out=rn[:, dm * 128:(dm + 1) * 128],
                                        in_=rt16[:, dm, h * 128:(h + 1) * 128], identity=identb[:])
                s1 = P.tile([128, 512], f32, tag=f"fs1{h}")
                nc.vector.tensor_tensor(out=s1[:], in0=rn[:], in1=outfeat[:, h, :], op=AL.add)
                s2 = P.tile([128, 512], f32, tag=f"fs2{h}")
                nc.vector.tensor_tensor(out=s2[:], in0=s1[:], in1=t2bb[:], op=AL.add)
                s3 = P.tile([128, 512], f32, tag=f"fs3{h}")
                nc.vector.tensor_scalar(out=s3[:], in0=s2[:], scalar1=0.0, scalar2=None, op0=AL.max)
                nc.sync.dma_start(refined_o.ap()[h], s3[:])

    nc.finalize()
    return nc


def _wrap_idx(ids):
    # ids: [512] int array -> wrapped [128, 32] int16 (idx k at [16g + k%16, k//16])
    w = ids.reshape(32, 16).T.astype(np.int16)  # [16, 32]
    return np.tile(w, (8, 1))


def _prep_core(obj, phr, pairs, params):
    (ws_w, ws_b, wo_w, wo_b, w_w, w_b, conv_w, conv_b,
     ln1_g, ln1_b, ln2_g, ln2_b, t1_w, t1_b, t2_w, t2_b) = params
    i_e = np.ascontiguousarray(pairs[:, 0]).astype(np.int64)
    j_e = np.ascontiguousarray(pairs[:, 1]).astype(np.int64)
    # uti[t, dp, (e,c)] = phr[128t+e, 128c+dp]
    phrT = np.ascontiguousarray(phr.T)                       # [512, 65536]
    uti = np.ascontiguousarray(
        phrT.reshape(4, 128, NT, 128).transpose(2, 1, 3, 0)  # [t, dp, e, c]
    ).reshape(NT, 128, 512).astype(np.float32)
    icol = i_e.reshape(NT, 128).T.astype(np.float32).copy()  # [128, NT]
    jcol = j_e.reshape(NT, 128).T.astype(np.float32).copy()
    gidx = np.zeros((128, NB * 64), np.int16)
    for b in range(NB):
        gidx[:, b * 64:b * 64 + 32] = _wrap_idx(i_e[b * 512:(b + 1) * 512])
        gidx[:, b * 64 + 32:b * 64 + 64] = _wrap_idx(j_e[b * 512:(b + 1) * 512])
    def chunks(v, n):  # bias vector [n*128] -> [128, n]
        return v.reshape(n, 128).T.astype(np.float32).copy()
    return {
        "uti": uti,
        "obj2": obj.reshape(2, 128, 512).astype(np.float32),
        "objT4": np.ascontiguousarray(obj.T).reshape(4, 128, 256).astype(np.float32),
        "wswT4": np.ascontiguousarray(ws_w.T).reshape(4, 128, 512).astype(np.float32),
        "wowT4": np.ascontiguousarray(wo_w.T).reshape(4, 128, 512).astype(np.float32),
        "convwT4": np.ascontiguousarray(conv_w.T).reshape(4, 128, 512).astype(np.float32),
        "t1wT4": np.ascontiguousarray(t1_w.T).reshape(4, 128, 1024).astype(np.float32),
        "t2wT8": np.ascontiguousarray(t2_w.T).reshape(8, 128, 512).astype(np.float32),
        "wsb4": chunks(ws_b, 4), "wob4": chunks(wo_b, 4),
        "wwT4": chunks(w_w[0], 4), "t1b8": chunks(t1_b, 8),
        "convb_b": np.tile(conv_b, (128, 1)).astype(np.float32),
        "t2b_b": np.tile(t2_b, (128, 1)).astype(np.float32),
        "ln1g_b": np.tile(ln1_g, (128, 1)).astype(np.float32),
        "ln1b_b": np.tile(ln1_b, (128, 1)).astype(np.float32),
        "ln2g_b": np.tile(ln2_g, (128, 1)).astype(np.float32),
        "ln2b_b": np.tile(ln2_b, (128, 1)).astype(np.float32),
        "wb_rep": np.full((128, 1), float(np.asarray(w_b).reshape(-1)[0]), np.float32),
        "icol": icol, "jcol": jcol, "gidx": gidx,
    }


def kernel(obj_feats, phr_feats, pair_idxs,
           ws_w, ws_b, wo_w, wo_b, w_w, w_b,
           conv_w, conv_b, ln1_g, ln1_b, ln2_g, ln2_b,
           t1_w, t1_b, t2_w, t2_b):
    global _CACHED_NC, LAST_RESULTS
    obj_feats = np.asarray(obj_feats, np.float32)
    phr_feats = np.asarray(phr_feats, np.float32)
    pairs = np.asarray(pair_idxs)
    params = tuple(np.asarray(p, np.float32) for p in
                   (ws_w, ws_b, wo_w, wo_b, w_w, w_b, conv_w, conv_b,
                    ln1_g, ln1_b, ln2_g, ln2_b, t1_w, t1_b, t2_w, t2_b))
    if _CACHED_NC is None:
        _CACHED_NC = _build_nc()
    nc = _CACHED_NC
    in_maps = [_prep_core(obj_feats[g], phr_feats[g], pairs[g], params) for g in range(B)]
    res = run_bass_kernel_spmd(nc, in_maps, core_ids=list(range(8)))
    LAST_RESULTS = res
    refined = np.stack([res.results[g]["refined_o"].reshape(256, 512) for g in range(B)])
    att = np.stack([res.results[g]["att_o"].reshape(256, 256) for g in range(B)])
    ctx = np.stack([res.results[g]["ctx_o"].reshape(256, 512) for g in range(B)])
    return refined, att, ctx


# revision 6
# speedup vs baseline: 1.0580x; 1.0580x over previous
"""Trainium2 Bass kernel for nn_ART_block (gnn_message_passing).
Data-parallel over B=8 graphs across 8 NeuronCores. Self-contained.
"""
import numpy as np
import ml_dtypes
import concourse.bass as bass
import concourse.mybir as mybir
import concourse.tile as tile
from concourse import bacc
from concourse.bass_utils import run_bass_kernel_spmd

f32 = mybir.dt.float32
bf16 = mybir.dt.bfloat16
i16 = mybir.dt.int16
AL = mybir.AluOpType
AF = mybir.ActivationFunctionType

B, N, E, D = 8, 256, 65536, 512
NT = E // 128          # 512 tiles of 128 edges
NB = NT // 4           # 128 batches of 4 tiles
EPS = 1e-5

_CACHED_NC = None
LAST_RESULTS = None


def _build_nc():
    nc = bacc.Bacc("TRN2", target_bir_lowering=False, debug=False, num_devices=8)
    # ---- inputs (per-core shard) ----
    uti = nc.dram_tensor("uti", [NT, 128, 512], f32, kind="ExternalInput")
    obj2 = nc.dram_tensor("obj2", [2, 128, 512], f32, kind="ExternalInput")
    objT4 = nc.dram_tensor("objT4", [4, 128, 256], f32, kind="ExternalInput")
    wswT4 = nc.dram_tensor("wswT4", [4, 128, 512], f32, kind="ExternalInput")
    wowT4 = nc.dram_tensor("wowT4", [4, 128, 512], f32, kind="ExternalInput")
    convwT4 = nc.dram_tensor("convwT4", [4, 128, 512], f32, kind="ExternalInput")
    t1wT4 = nc.dram_tensor("t1wT4", [4, 128, 1024], f32, kind="ExternalInput")
    t2wT8 = nc.dram_tensor("t2wT8", [8, 128, 512], f32, kind="ExternalInput")
    wsb4 = nc.dram_tensor("wsb4", [128, 4], f32, kind="ExternalInput")
    wob4 = nc.dram_tensor("wob4", [128, 4], f32, kind="ExternalInput")
    wwT4 = nc.dram_tensor("wwT4", [128, 4], f32, kind="ExternalInput")
    t1b8 = nc.dram_tensor("t1b8", [128, 8], f32, kind="ExternalInput")
    convb_b = nc.dram_tensor("convb_b", [128, 512], f32, kind="ExternalInput")
    t2b_b = nc.dram_tensor("t2b_b", [128, 512], f32, kind="ExternalInput")
    ln1g_b = nc.dram_tensor("ln1g_b", [128, 512], f32, kind="ExternalInput")
    ln1b_b = nc.dram_tensor("ln1b_b", [128, 512], f32, kind="ExternalInput")
    ln2g_b = nc.dram_tensor("ln2g_b", [128, 512], f32, kind="ExternalInput")
    ln2b_b = nc.dram_tensor("ln2b_b", [128, 512], f32, kind="ExternalInput")
    wb_rep = nc.dram_tensor("wb_rep", [128, 1], f32, kind="ExternalInput")
    icol = nc.dram_tensor("icol", [128, NT], f32, kind="ExternalInput")
    jcol = nc.dram_tensor("jcol", [128, NT], f32, kind="ExternalInput")
    gidx = nc.dram_tensor("gidx", [128, NB * 64], i16, kind="ExternalInput")
    # ---- outputs ----
    refined_o = nc.dram_tensor("refined_o", [2, 128, 512], f32, kind="ExternalOutput")
    att_o = nc.dram_tensor("att_o", [2, 128, 256], f32, kind="ExternalOutput")
    ctx_o = nc.dram_tensor("ctx_o", [2, 128, 512], f32, kind="ExternalOutput")

    # ---- inline consts ----
    iota_c = nc.inline_tensor(np.tile(np.arange(256), (128, 1)).astype(ml_dtypes.bfloat16), name="iota_c")
    ones_c = nc.inline_tensor(np.ones((128, 1), dtype=ml_dtypes.bfloat16), name="ones_c")
    identb_c = nc.inline_tensor(np.eye(128).astype(ml_dtypes.bfloat16), name="identb_c")
    identf_c = nc.inline_tensor(np.eye(128, dtype=np.float32), name="identf_c")
    eye_np = np.zeros((2, 128, 256), np.float32)
    for h in range(2):
        for p in range(128):
            eye_np[h, p, h * 128 + p] = 10000.0
    eye_c = nc.inline_tensor(eye_np, name="eye_c")

    with tile.TileContext(nc) as tc:
        with (
            tc.tile_pool(name="pers", bufs=1) as P,           # persistent SBUF
            tc.tile_pool(name="loop", bufs=3) as L,           # streaming tiles
            tc.tile_pool(name="psA", bufs=1, space="PSUM") as PSA,   # att banks
            tc.tile_pool(name="psF", bufs=2, space="PSUM") as PSF,   # atten_f
            tc.tile_pool(name="psW", bufs=2, space="PSUM") as PSW,   # prologue/epilogue work
        ):
            # ---------- preload params ----------
            def load(pool, dram, shape, dtype=f32):
                t = pool.tile(shape, dtype, name=dram.name + "_sb")
                nc.sync.dma_start(t[:], dram.ap())
                return t

            iota_t = load(P, iota_c, [128, 256], bf16)
            ones_t = load(P, ones_c, [128, 1], bf16)
            identb = load(P, identb_c, [128, 128], bf16)
            eye_t = P.tile([128, 2, 256], f32)
            nc.sync.dma_start(eye_t[:], eye_c.ap().rearrange("h p b -> p h b"))
            objs = P.tile([128, 2, 512], f32)
            nc.sync.dma_start(objs[:], obj2.ap().rearrange("h p e -> p h e"))
            objT = P.tile([128, 4, 256], f32)
            nc.sync.dma_start(objT[:], objT4.ap().rearrange("k p a -> p k a"))
            wswT = P.tile([128, 4, 512], f32)
            nc.sync.dma_start(wswT[:], wswT4.ap().rearrange("k p a -> p k a"))
            wowT = P.tile([128, 4, 512], f32)
            nc.sync.dma_start(wowT[:], wowT4.ap().rearrange("k p a -> p k a"))



            wsb = load(P, wsb4, [128, 4])
            wob = load(P, wob4, [128, 4])
            wwT = load(P, wwT4, [128, 4])
            t1b = load(P, t1b8, [128, 8])
            convb = load(P, convb_b, [128, 512])
            t2bb = load(P, t2b_b, [128, 512])
            ln1g = load(P, ln1g_b, [128, 512])
            ln1b = load(P, ln1b_b, [128, 512])
            ln2g = load(P, ln2g_b, [128, 512])
            ln2b = load(P, ln2b_b, [128, 512])
            wbr = load(P, wb_rep, [128, 1])
            icolt = load(P, icol, [128, NT])
            jcolt = load(P, jcol, [128, NT])
            gidxt = load(P, gidx, [128, NB * 64], i16)

            # ---------- prologue: gather tables s'T, oT (interleaved bf16) ----------
            tabS = P.tile([128, 256, 4], bf16)
            tabO = P.tile([128, 256, 4], bf16)
            for c in range(4):
                mm = PSW.tile([128, 256], f32, space="PSUM", tag="w")
                for k in range(4):
                    nc.tensor.matmul(out=mm[:], lhsT=wswT[:, k, c * 128:(c + 1) * 128],
                                     rhs=objT[:, k, :], start=(k == 0), stop=(k == 3))
                nc.vector.tensor_scalar(out=tabS[:, :, c], in0=mm[:],
                                        scalar1=wsb[:, c:c + 1], scalar2=wwT[:, c:c + 1],
                                        op0=AL.add, op1=AL.mult)
                mo = PSW.tile([128, 256], f32, space="PSUM", tag="w")
                for k in range(4):
                    nc.tensor.matmul(out=mo[:], lhsT=wowT[:, k, c * 128:(c + 1) * 128],
                                     rhs=objT[:, k, :], start=(k == 0), stop=(k == 3))
                nc.vector.tensor_scalar(out=tabO[:, :, c], in0=mo[:],
                                        scalar1=wob[:, c:c + 1], scalar2=None, op0=AL.add)

            # ---------- prologue: conv path c16 = relu(LN1(obj) @ conv_w.T) ----------
            def layernorm(src_h, g_t, b_t, out16_h, tag):
                # src_h: [128,512] f32 SBUF; writes bf16 normalized out
                s1 = P.tile([128, 1], f32, tag=tag + "s1")
                nc.vector.tensor_reduce(out=s1[:], in_=src_h[:], axis=mybir.AxisListType.X, op=AL.add)
                nm = P.tile([128, 1], f32, tag=tag + "nm")
                nc.vector.tensor_scalar(out=nm[:], in0=s1[:], scalar1=-1.0 / 512, scalar2=None, op0=AL.mult)
                xm = P.tile([128, 512], f32, tag="lnxm", name="lnxm")
                nc.scalar.activation(out=xm[:], in_=src_h[:], func=AF.Identity, bias=nm[:], scale=1.0)
                ssq = P.tile([128, 1], f32, tag=tag + "sq")
                junk = P.tile([128, 512], bf16, tag="lnjk", name="lnjk")
                nc.vector.scalar_tensor_tensor(out=junk[:], in0=xm[:], scalar=1.0, in1=xm[:],
                                               op0=AL.mult, op1=AL.mult, accum_out=ssq[:])
                var = P.tile([128, 1], f32, tag=tag + "vr")
                nc.vector.tensor_scalar(out=var[:], in0=ssq[:], scalar1=1.0 / 512, scalar2=EPS,
                                        op0=AL.mult, op1=AL.add)
                sd = P.tile([128, 1], f32, tag=tag + "sd")
                nc.scalar.activation(out=sd[:], in_=var[:], func=AF.Sqrt)
                rs = P.tile([128, 1], f32, tag=tag + "rs")
                nc.vector.reciprocal(rs[:], sd[:])
                xn = P.tile([128, 512], f32, tag="lnxn", name="lnxn")
                nc.scalar.activation(out=xn[:], in_=xm[:], func=AF.Copy, scale=rs[:])
                xg = P.tile([128, 512], f32, tag="lnxg", name="lnxg")
                nc.vector.tensor_tensor(out=xg[:], in0=xn[:], in1=g_t[:], op=AL.mult)
                xf = P.tile([128, 512], f32, tag="lnxf", name="lnxf")
                nc.vector.tensor_tensor(out=xf[:], in0=xg[:], in1=b_t[:], op=AL.add)
                nc.vector.tensor_copy(out16_h[:], xf[:])
                return xf

            ln1_16 = [P.tile([128, 512], bf16, tag=f"l1_{h}", name=f"l1_{h}") for h in range(2)]
            for h in range(2):
                layernorm(objs[:, h, :], ln1g, ln1b, ln1_16[h], f"ln1{h}")
            # transpose LN1 -> lnT [4][128dp, 256b] bf16
            lnT = P.tile([128, 4, 256], bf16)
            for dk in range(4):
                tp = PSW.tile([128, 256], bf16, space="PSUM", tag="w")
                for h in range(2):
                    nc.tensor.transpose(out=tp[:, h * 128:(h + 1) * 128],
                                        in_=ln1_16[h][:, dk * 128:(dk + 1) * 128], identity=identb[:])
                nc.vector.tensor_copy(lnT[:, dk, :], tp[:])
            convwT16 = P.tile([128, 4, 512], bf16)
            nc.gpsimd.dma_start(convwT16[:], convwT4.ap().rearrange("k p a -> p k a"))
            c16 = P.tile([128, 2, 512], bf16)
            for h in range(2):
                cp = PSW.tile([128, 512], f32, space="PSUM", tag="w")
                for dk in range(4):
                    nc.tensor.matmul(out=cp[:], lhsT=lnT[:, dk, h * 128:(h + 1) * 128],
                                     rhs=convwT16[:, dk, :], start=(dk == 0), stop=(dk == 3))
                cb = P.tile([128, 512], f32, tag=f"cb{h}")
                nc.vector.tensor_tensor(out=cb[:], in0=cp[:], in1=convb[:], op=AL.add)
                nc.vector.tensor_scalar(out=c16[:, h, :], in0=cb[:], scalar1=0.0, scalar2=None, op0=AL.max)

            # ---------- att PSUM init (start=True zero matmul) ----------
            att_ps = [PSA.tile([128, 256], f32, space="PSUM", tag=f"att{h}", name=f"att{h}") for h in range(2)]
            zero16 = P.tile([128, 256], bf16)
            nc.vector.memset(zero16[:], 0.0)
            for h in range(2):
                nc.tensor.matmul(out=att_ps[h][:], lhsT=zero16[:, :128], rhs=zero16[:],
                                 start=True, stop=False)

            # ---------- streaming: fully unrolled static ----------
            def batch_body(bi):
                g1 = L.tile([128, 512, 4], bf16, tag="g1")
                g2 = L.tile([128, 512, 4], bf16, tag="g2")
                nc.gpsimd.ap_gather(out_ap=g1[:], in_ap=tabS[:], idxs_ap=gidxt[:, bi * 64:bi * 64 + 32],
                                    channels=128, num_elems=256, d=4, num_idxs=512)
                nc.gpsimd.ap_gather(out_ap=g2[:], in_ap=tabO[:], idxs_ap=gidxt[:, bi * 64 + 32:bi * 64 + 64],
                                    channels=128, num_elems=256, d=4, num_idxs=512)
                for k in range(4):
                    t = bi * 4 + k
                    u16 = L.tile([128, 512], bf16, tag="u16")
                    nc.gpsimd.dma_start(u16[:], uti.ap()[t])
                    mi = L.tile([128, 128, 4], bf16, tag="mi")
                    nc.vector.tensor_tensor(out=mi[:].rearrange("p a b -> p (a b)"),
                                            in0=g1[:, k * 128:(k + 1) * 128, :].rearrange("p a b -> p (a b)"),
                                            in1=g2[:, k * 128:(k + 1) * 128, :].rearrange("p a b -> p (a b)"),
                                            op=AL.mult)
                    pr = L.tile([128, 128, 4], bf16, tag="pr")
                    nc.vector.tensor_tensor(out=pr[:].rearrange("p a b -> p (a b)"),
                                            in0=mi[:].rearrange("p a b -> p (a b)"),
                                            in1=u16[:].rearrange("p e -> p e"), op=AL.mult)
                    fps = PSF.tile([128, 1], f32, space="PSUM", tag="fps")
                    for c in range(4):
                        nc.tensor.matmul(out=fps[:], lhsT=pr[:, :, c], rhs=ones_t[:],
                                         start=(c == 0), stop=(c == 3))
                    fb = L.tile([128, 1], f32, tag="fb")
                    nc.vector.tensor_scalar(out=fb[:], in0=fps[:], scalar1=wbr[:], scalar2=None, op0=AL.add)
                    ohi = L.tile([128, 256], bf16, tag="ohi")
                    nc.vector.tensor_scalar(out=ohi[:], in0=iota_t[:], scalar1=icolt[:, t:t + 1],
                                            scalar2=None, op0=AL.is_equal)
                    xt = L.tile([128, 256], bf16, tag="xt")
                    nc.vector.tensor_scalar(out=xt[:], in0=iota_t[:], scalar1=jcolt[:, t:t + 1],
                                            scalar2=fb[:], op0=AL.is_equal, op1=AL.mult)
                    for h in range(2):
                        nc.tensor.matmul(out=att_ps[h][:], lhsT=ohi[:, h * 128:(h + 1) * 128],
                                         rhs=xt[:], start=False, stop=False)

            for bi in range(NB):
                batch_body(bi)

            # close psum accumulation
            for h in range(2):
                nc.tensor.matmul(out=att_ps[h][:], lhsT=zero16[:, :128], rhs=zero16[:],
                                 start=False, stop=True)

            # ---------- epilogue ----------
            att16 = P.tile([128, 2, 256], bf16)
            attf = P.tile([128, 2, 256], f32)
            for h in range(2):
                am = P.tile([128, 256], f32, tag=f"am{h}")
                nc.vector.tensor_tensor(out=am[:], in0=att_ps[h][:], in1=eye_t[:, h, :], op=AL.subtract)
                mx = P.tile([128, 1], f32, tag=f"mx{h}")
                nc.vector.tensor_reduce(out=mx[:], in_=am[:], axis=mybir.AxisListType.X, op=AL.max)
                nmx = P.tile([128, 1], f32, tag=f"nmx{h}")
                nc.vector.tensor_scalar(out=nmx[:], in0=mx[:], scalar1=-1.0, scalar2=None, op0=AL.mult)
                ex = P.tile([128, 256], f32, tag=f"ex{h}")
                sm = P.tile([128, 1], f32, tag=f"sm{h}")
                nc.scalar.activation(out=ex[:], in_=am[:], func=AF.Exp, bias=nmx[:], scale=1.0,
                                     accum_out=sm[:])
                rs = P.tile([128, 1], f32, tag=f"rsm{h}")
                nc.vector.reciprocal(rs[:], sm[:])
                nc.scalar.activation(out=attf[:, h, :], in_=ex[:], func=AF.Copy, scale=rs[:])
                nc.vector.tensor_copy(att16[:, h, :], attf[:, h, :])
                nc.sync.dma_start(att_o.ap()[h], attf[:, h, :])
            # attT for ctx matmul
            attT = P.tile([128, 2, 256], bf16)  # [128b, bk, 256a]
            for bk in range(2):
                tp = PSW.tile([128, 256], bf16, space="PSUM", tag="w")
                for h in range(2):
                    nc.tensor.transpose(out=tp[:, h * 128:(h + 1) * 128],
                                        in_=att16[:, h, bk * 128:(bk + 1) * 128], identity=identb[:])
                nc.vector.tensor_copy(attT[:, bk, :], tp[:])
            outfeat = P.tile([128, 2, 512], f32)
            ctxf = P.tile([128, 2, 512], f32)
            for h in range(2):
                cxp = PSW.tile([128, 512], f32, space="PSUM", tag="w")
                for bk in range(2):
                    nc.tensor.matmul(out=cxp[:], lhsT=attT[:, bk, h * 128:(h + 1) * 128],
                                     rhs=c16[:, bk, :], start=(bk == 0), stop=(bk == 1))
                nc.vector.tensor_copy(ctxf[:, h, :], cxp[:])
                nc.sync.dma_start(ctx_o.ap()[h], ctxf[:, h, :])
                nc.vector.tensor_tensor(out=outfeat[:, h, :], in0=cxp[:], in1=objs[:, h, :], op=AL.add)
            # LN2 + MLP
            h16 = [P.tile([128, 512], bf16, tag=f"h16_{h}", name=f"h16_{h}") for h in range(2)]
            for h in range(2):
                layernorm(outfeat[:, h, :], ln2g, ln2b, h16[h], f"ln2{h}")
            hT = P.tile([128, 4, 256], bf16)
            for dk in range(4):
                tp = PSW.tile([128, 256], bf16, space="PSUM", tag="w")
                for h in range(2):
                    nc.tensor.transpose(out=tp[:, h * 128:(h + 1) * 128],
                                        in_=h16[h][:, dk * 128:(dk + 1) * 128], identity=identb[:])
                nc.vector.tensor_copy(hT[:, dk, :], tp[:])
            t1wT16 = P.tile([128, 4, 1024], bf16)
            nc.gpsimd.dma_start(t1wT16[:], t1wT4.ap().rearrange("k p a -> p k a"))
            t2wT16 = P.tile([128, 8, 512], bf16)
            nc.gpsimd.dma_start(t2wT16[:], t2wT8.ap().rearrange("k p a -> p k a"))
            h1r = P.tile([128, 8, 256], bf16)
            for fm in range(8):
                hp = PSW.tile([128, 256], f32, space="PSUM", tag="w")
                for dk in range(4):
                    nc.tensor.matmul(out=hp[:], lhsT=t1wT16[:, dk, fm * 128:(fm + 1) * 128],
                                     rhs=hT[:, dk, :], start=(dk == 0), stop=(dk == 3))
                nc.scalar.activation(out=h1r[:, fm, :], in_=hp[:], func=AF.Relu, bias=t1b[:, fm:fm + 1], scale=1.0)
            rt16 = P.tile([128, 4, 256], bf16)
            for dm in range(4):
                rp = PSW.tile([128, 256], f32, space="PSUM", tag="w")
                for fk in range(8):
                    nc.tensor.matmul(out=rp[:], lhsT=t2wT16[:, fk, dm * 128:(dm + 1) * 128],
                                     rhs=h1r[:, fk, :], start=(fk == 0), stop=(fk == 7))
                nc.vector.tensor_copy(rt16[:, dm, :], rp[:])
            for h in range(2):
                rn = PSW.tile([128, 512], bf16, space="PSUM", tag="w")
                for dm in range(4):
                    nc.tensor.transpose(out=rn[:, dm * 128:(dm + 1) * 128],
                                        in_=rt16[:, dm, h * 128:(h + 1) * 128], identity=identb[:])
                s1 = P.tile([128, 512], f32, tag=f"fs1{h}")
                nc.vector.tensor_tensor(out=s1[:], in0=rn[:], in1=outfeat[:, h, :], op=AL.add)
                s2 = P.tile([128, 512], f32, tag=f"fs2{h}")
                nc.vector.tensor_tensor(out=s2[:], in0=s1[:], in1=t2bb[:], op=AL.add)
                s3 = P.tile([128, 512], f32, tag=f"fs3{h}")
                nc.vector.tensor_scalar(out=s3[:], in0=s2[:], scalar1=0.0, scalar2=None, op0=AL.max)
                nc.sync.dma_start(refined_o.ap()[h], s3[:])

    nc.finalize()
    return nc


def _wrap_idx(ids):
    # ids: [512] int array -> wrapped [128, 32] int16 (idx k at [16g + k%16, k//16])
    w = ids.reshape(32, 16).T.astype(np.int16)  # [16, 32]
    return np.tile(w, (8, 1))


def _prep_core(obj, phr, pairs, params):
    (ws_w, ws_b, wo_w, wo_b, w_w, w_b, conv_w, conv_b,
     ln1_g, ln1_b, ln2_g, ln2_b, t1_w, t1_b, t2_w, t2_b) = params
    i_e = np.ascontiguousarray(pairs[:, 0]).astype(np.int64)
    j_e = np.ascontiguousarray(pairs[:, 1]).astype(np.int64)
    # uti[t, dp, (e,c)] = phr[128t+e, 128c+dp]
    phrT = np.ascontiguousarray(phr.T)                       # [512, 65536]
    uti = np.ascontiguousarray(
        phrT.reshape(4, 128, NT, 128).transpose(2, 1, 3, 0)  # [t, dp, e, c]
    ).reshape(NT, 128, 512).astype(np.float32)
    icol = i_e.reshape(NT, 128).T.astype(np.float32).copy()  # [128, NT]
    jcol = j_e.reshape(NT, 128).T.astype(np.float32).copy()
    gidx = np.zeros((128, NB * 64), np.int16)
    for b in range(NB):
        gidx[:, b * 64:b * 64 + 32] = _wrap_idx(i_e[b * 512:(b + 1) * 512])
        gidx[:, b * 64 + 32:b * 64 + 64] = _wrap_idx(j_e[b * 512:(b + 1) * 512])
    def chunks(v, n):  # bias vector [n*128] -> [128, n]
        return v.reshape(n, 128).T.astype(np.float32).copy()
    return {
        "uti": uti,
        "obj2": obj.reshape(2, 128, 512).astype(np.float32),
        "objT4": np.ascontiguousarray(obj.T).reshape(4, 128, 256).astype(np.float32),
        "wswT4": np.ascontiguousarray(ws_w.T).reshape(4, 128, 512).astype(np.float32),
        "wowT4": np.ascontiguousarray(wo_w.T).reshape(4, 128, 512).astype(np.float32),
        "convwT4": np.ascontiguousarray(conv_w.T).reshape(4, 128, 512).astype(np.float32),
        "t1wT4": np.ascontiguousarray(t1_w.T).reshape(4, 128, 1024).astype(np.float32),
        "t2wT8": np.ascontiguousarray(t2_w.T).reshape(8, 128, 512).astype(np.float32),
        "wsb4": chunks(ws_b, 4), "wob4": chunks(wo_b, 4),
        "wwT4": chunks(w_w[0], 4), "t1b8": chunks(t1_b, 8),
        "convb_b": np.tile(conv_b, (128, 1)).astype(np.float32),
        "t2b_b": np.tile(t2_b, (128, 1)).astype(np.float32),
        "ln1g_b": np.tile(ln1_g, (128, 1)).astype(np.float32),
        "ln1b_b": np.tile(ln1_b, (128, 1)).astype(np.float32),
        "ln2g_b": np.tile(ln2_g, (128, 1)).astype(np.float32),
        "ln2b_b": np.tile(ln2_b, (128, 1)).astype(np.float32),
        "wb_rep": np.full((128, 1), float(np.asarray(w_b).reshape(-1)[0]), np.float32),
        "icol": icol, "jcol": jcol, "gidx": gidx,
    }


def kernel(obj_feats, phr_feats, pair_idxs,
           ws_w, ws_b, wo_w, wo_b, w_w, w_b,
           conv_w, conv_b, ln1_g, ln1_b, ln2_g, ln2_b,
           t1_w, t1_b, t2_w, t2_b):
    global _CACHED_NC, LAST_RESULTS
    obj_feats = np.asarray(obj_feats, np.float32)
    phr_feats = np.asarray(phr_feats, np.float32)
    pairs = np.asarray(pair_idxs)
    params = tuple(np.asarray(p, np.float32) for p in
                   (ws_w, ws_b, wo_w, wo_b, w_w, w_b, conv_w, conv_b,
                    ln1_g, ln1_b, ln2_g, ln2_b, t1_w, t1_b, t2_w, t2_b))
    if _CACHED_NC is None:
        _CACHED_NC = _build_nc()
    nc = _CACHED_NC
    in_maps = [_prep_core(obj_feats[g], phr_feats[g], pairs[g], params) for g in range(B)]
    res = run_bass_kernel_spmd(nc, in_maps, core_ids=list(range(8)))
    LAST_RESULTS = res
    refined = np.stack([res.results[g]["refined_o"].reshape(256, 512) for g in range(B)])
    att = np.stack([res.results[g]["att_o"].reshape(256, 256) for g in range(B)])
    ctx = np.stack([res.results[g]["ctx_o"].reshape(256, 512) for g in range(B)])
    return refined, att, ctx


# revision 7
# speedup vs baseline: 1.0870x; 1.0274x over previous
"""Trainium2 Bass kernel for nn_ART_block (gnn_message_passing).
Data-parallel over B=8 graphs across 8 NeuronCores. Self-contained.
"""
import numpy as np
import ml_dtypes
import concourse.bass as bass
import concourse.mybir as mybir
import concourse.tile as tile
from concourse import bacc
from concourse.bass_utils import run_bass_kernel_spmd

f32 = mybir.dt.float32
bf16 = mybir.dt.bfloat16
i16 = mybir.dt.int16
AL = mybir.AluOpType
AF = mybir.ActivationFunctionType

B, N, E, D = 8, 256, 65536, 512
NT = E // 128          # 512 tiles of 128 edges
NB = NT // 4           # 128 batches of 4 tiles
EPS = 1e-5

_CACHED_NC = None
LAST_RESULTS = None


def _build_nc():
    nc = bacc.Bacc("TRN2", target_bir_lowering=False, debug=False, num_devices=8)
    # ---- inputs (per-core shard) ----
    uti = nc.dram_tensor("uti", [NT, 128, 512], f32, kind="ExternalInput")
    obj2 = nc.dram_tensor("obj2", [2, 128, 512], f32, kind="ExternalInput")
    objT4 = nc.dram_tensor("objT4", [4, 128, 256], f32, kind="ExternalInput")
    wswT4 = nc.dram_tensor("wswT4", [4, 128, 512], f32, kind="ExternalInput")
    wowT4 = nc.dram_tensor("wowT4", [4, 128, 512], f32, kind="ExternalInput")
    convwT4 = nc.dram_tensor("convwT4", [4, 128, 512], f32, kind="ExternalInput")
    t1wT4 = nc.dram_tensor("t1wT4", [4, 128, 1024], f32, kind="ExternalInput")
    t2wT8 = nc.dram_tensor("t2wT8", [8, 128, 512], f32, kind="ExternalInput")
    wsb4 = nc.dram_tensor("wsb4", [128, 4], f32, kind="ExternalInput")
    wob4 = nc.dram_tensor("wob4", [128, 4], f32, kind="ExternalInput")
    wwT4 = nc.dram_tensor("wwT4", [128, 4], f32, kind="ExternalInput")
    t1b8 = nc.dram_tensor("t1b8", [128, 8], f32, kind="ExternalInput")
    convb_b = nc.dram_tensor("convb_b", [128, 512], f32, kind="ExternalInput")
    t2b_b = nc.dram_tensor("t2b_b", [128, 512], f32, kind="ExternalInput")
    ln1g_b = nc.dram_tensor("ln1g_b", [128, 512], f32, kind="ExternalInput")
    ln1b_b = nc.dram_tensor("ln1b_b", [128, 512], f32, kind="ExternalInput")
    ln2g_b = nc.dram_tensor("ln2g_b", [128, 512], f32, kind="ExternalInput")
    ln2b_b = nc.dram_tensor("ln2b_b", [128, 512], f32, kind="ExternalInput")
    wb_rep = nc.dram_tensor("wb_rep", [128, 1], f32, kind="ExternalInput")
    icol = nc.dram_tensor("icol", [128, NT], f32, kind="ExternalInput")
    jcol = nc.dram_tensor("jcol", [128, NT], f32, kind="ExternalInput")
    gidx = nc.dram_tensor("gidx", [128, NB * 64], i16, kind="ExternalInput")
    # ---- outputs ----
    refined_o = nc.dram_tensor("refined_o", [2, 128, 512], f32, kind="ExternalOutput")
    att_o = nc.dram_tensor("att_o", [2, 128, 256], f32, kind="ExternalOutput")
    ctx_o = nc.dram_tensor("ctx_o", [2, 128, 512], f32, kind="ExternalOutput")

    # ---- inline consts ----
    iota_c = nc.inline_tensor(np.tile(np.arange(256), (128, 1)).astype(ml_dtypes.bfloat16), name="iota_c")
    ones_c = nc.inline_tensor(np.ones((128, 1), dtype=ml_dtypes.bfloat16), name="ones_c")
    identb_c = nc.inline_tensor(np.eye(128).astype(ml_dtypes.bfloat16), name="identb_c")
    identf_c = nc.inline_tensor(np.eye(128, dtype=np.float32), name="identf_c")
    eye_np = np.zeros((2, 128, 256), np.float32)
    for h in range(2):
        for p in range(128):
            eye_np[h, p, h * 128 + p] = 10000.0
    eye_c = nc.inline_tensor(eye_np, name="eye_c")

    with tile.TileContext(nc) as tc:
        with (
            tc.tile_pool(name="pers", bufs=1) as P,           # persistent SBUF
            tc.tile_pool(name="loop", bufs=3) as L,           # streaming tiles
            tc.tile_pool(name="psA", bufs=1, space="PSUM") as PSA,   # att banks
            tc.tile_pool(name="psF", bufs=2, space="PSUM") as PSF,   # atten_f
            tc.tile_pool(name="psW", bufs=2, space="PSUM") as PSW,   # prologue/epilogue work
        ):
            # ---------- preload params ----------
            def load(pool, dram, shape, dtype=f32):
                t = pool.tile(shape, dtype, name=dram.name + "_sb")
                nc.sync.dma_start(t[:], dram.ap())
                return t

            iota_t = load(P, iota_c, [128, 256], bf16)
            ones_t = load(P, ones_c, [128, 1], bf16)
            identb = load(P, identb_c, [128, 128], bf16)
            eye_t = P.tile([128, 2, 256], f32)
            nc.sync.dma_start(eye_t[:], eye_c.ap().rearrange("h p b -> p h b"))
            objs = P.tile([128, 2, 512], f32)
            nc.sync.dma_start(objs[:], obj2.ap().rearrange("h p e -> p h e"))
            objT = P.tile([128, 4, 256], f32)
            nc.sync.dma_start(objT[:], objT4.ap().rearrange("k p a -> p k a"))
            wswT = P.tile([128, 4, 512], f32)
            nc.sync.dma_start(wswT[:], wswT4.ap().rearrange("k p a -> p k a"))
            wowT = P.tile([128, 4, 512], f32)
            nc.sync.dma_start(wowT[:], wowT4.ap().rearrange("k p a -> p k a"))



            wsb = load(P, wsb4, [128, 4])
            wob = load(P, wob4, [128, 4])
            wwT = load(P, wwT4, [128, 4])
            t1b = load(P, t1b8, [128, 8])
            convb = load(P, convb_b, [128, 512])
            t2bb = load(P, t2b_b, [128, 512])
            ln1g = load(P, ln1g_b, [128, 512])
            ln1b = load(P, ln1b_b, [128, 512])
            ln2g = load(P, ln2g_b, [128, 512])
            ln2b = load(P, ln2b_b, [128, 512])
            wbr = load(P, wb_rep, [128, 1])
            icolt = load(P, icol, [128, NT])
            jcolt = load(P, jcol, [128, NT])
            gidxt = load(P, gidx, [128, NB * 64], i16)

            # ---------- prologue: gather tables s'T, oT (interleaved bf16) ----------
            tabS = P.tile([128, 256, 4], bf16)
            tabO = P.tile([128, 256, 4], bf16)
            for c in range(4):
                mm = PSW.tile([128, 256], f32, space="PSUM", tag="w")
                for k in range(4):
                    nc.tensor.matmul(out=mm[:], lhsT=wswT[:, k, c * 128:(c + 1) * 128],
                                     rhs=objT[:, k, :], start=(k == 0), stop=(k == 3))
                nc.vector.tensor_scalar(out=tabS[:, :, c], in0=mm[:],
                                        scalar1=wsb[:, c:c + 1], scalar2=wwT[:, c:c + 1],
                                        op0=AL.add, op1=AL.mult)
                mo = PSW.tile([128, 256], f32, space="PSUM", tag="w")
                for k in range(4):
                    nc.tensor.matmul(out=mo[:], lhsT=wowT[:, k, c * 128:(c + 1) * 128],
                                     rhs=objT[:, k, :], start=(k == 0), stop=(k == 3))
                nc.vector.tensor_scalar(out=tabO[:, :, c], in0=mo[:],
                                        scalar1=wob[:, c:c + 1], scalar2=None, op0=AL.add)

            # ---------- prologue: conv path c16 = relu(LN1(obj) @ conv_w.T) ----------
            def layernorm(src_h, g_t, b_t, out16_h, tag):
                # src_h: [128,512] f32 SBUF; writes bf16 normalized out
                s1 = P.tile([128, 1], f32, tag=tag + "s1")
                nc.vector.tensor_reduce(out=s1[:], in_=src_h[:], axis=mybir.AxisListType.X, op=AL.add)
                nm = P.tile([128, 1], f32, tag=tag + "nm")
                nc.vector.tensor_scalar(out=nm[:], in0=s1[:], scalar1=-1.0 / 512, scalar2=None, op0=AL.mult)
                xm = P.tile([128, 512], f32, tag="lnxm", name="lnxm")
                nc.scalar.activation(out=xm[:], in_=src_h[:], func=AF.Identity, bias=nm[:], scale=1.0)
                ssq = P.tile([128, 1], f32, tag=tag + "sq")
                junk = P.tile([128, 512], bf16, tag="lnjk", name="lnjk")
                nc.vector.scalar_tensor_tensor(out=junk[:], in0=xm[:], scalar=1.0, in1=xm[:],
                                               op0=AL.mult, op1=AL.mult, accum_out=ssq[:])
                var = P.tile([128, 1], f32, tag=tag + "vr")
                nc.vector.tensor_scalar(out=var[:], in0=ssq[:], scalar1=1.0 / 512, scalar2=EPS,
                                        op0=AL.mult, op1=AL.add)
                sd = P.tile([128, 1], f32, tag=tag + "sd")
                nc.scalar.activation(out=sd[:], in_=var[:], func=AF.Sqrt)
                rs = P.tile([128, 1], f32, tag=tag + "rs")
                nc.vector.reciprocal(rs[:], sd[:])
                xn = P.tile([128, 512], f32, tag="lnxn", name="lnxn")
                nc.scalar.activation(out=xn[:], in_=xm[:], func=AF.Copy, scale=rs[:])
                xg = P.tile([128, 512], f32, tag="lnxg", name="lnxg")
                nc.vector.tensor_tensor(out=xg[:], in0=xn[:], in1=g_t[:], op=AL.mult)
                xf = P.tile([128, 512], f32, tag="lnxf", name="lnxf")
                nc.vector.tensor_tensor(out=xf[:], in0=xg[:], in1=b_t[:], op=AL.add)
                nc.vector.tensor_copy(out16_h[:], xf[:])
                return xf

            ln1_16 = [P.tile([128, 512], bf16, tag=f"l1_{h}", name=f"l1_{h}") for h in range(2)]
            for h in range(2):
                layernorm(objs[:, h, :], ln1g, ln1b, ln1_16[h], f"ln1{h}")
            # transpose LN1 -> lnT [4][128dp, 256b] bf16
            lnT = P.tile([128, 4, 256], bf16)
            for dk in range(4):
                tp = PSW.tile([128, 256], bf16, space="PSUM", tag="w")
                for h in range(2):
                    nc.tensor.transpose(out=tp[:, h * 128:(h + 1) * 128],
                                        in_=ln1_16[h][:, dk * 128:(dk + 1) * 128], identity=identb[:])
                nc.vector.tensor_copy(lnT[:, dk, :], tp[:])
            convwT16 = P.tile([128, 4, 512], bf16)
            nc.gpsimd.dma_start(convwT16[:], convwT4.ap().rearrange("k p a -> p k a"))
            c16 = P.tile([128, 2, 512], bf16)
            for h in range(2):
                cp = PSW.tile([128, 512], f32, space="PSUM", tag="w")
                for dk in range(4):
                    nc.tensor.matmul(out=cp[:], lhsT=lnT[:, dk, h * 128:(h + 1) * 128],
                                     rhs=convwT16[:, dk, :], start=(dk == 0), stop=(dk == 3))
                cb = P.tile([128, 512], f32, tag=f"cb{h}")
                nc.vector.tensor_tensor(out=cb[:], in0=cp[:], in1=convb[:], op=AL.add)
                nc.vector.tensor_scalar(out=c16[:, h, :], in0=cb[:], scalar1=0.0, scalar2=None, op0=AL.max)

            # ---------- att PSUM init (start=True zero matmul) ----------
            att_ps = [PSA.tile([128, 256], f32, space="PSUM", tag=f"att{h}", name=f"att{h}") for h in range(2)]
            zero16 = P.tile([128, 256], bf16)
            nc.vector.memset(zero16[:], 0.0)
            for h in range(2):
                nc.tensor.matmul(out=att_ps[h][:], lhsT=zero16[:, :128], rhs=zero16[:],
                                 start=True, stop=False)

            # ---------- streaming: fully unrolled static ----------
            def batch_body(bi):
                g1 = L.tile([128, 512, 4], bf16, tag="g1", bufs=3)
                g2 = L.tile([128, 512, 4], bf16, tag="g2", bufs=3)
                nc.gpsimd.ap_gather(out_ap=g1[:], in_ap=tabS[:], idxs_ap=gidxt[:, bi * 64:bi * 64 + 32],
                                    channels=128, num_elems=256, d=4, num_idxs=512)
                nc.gpsimd.ap_gather(out_ap=g2[:], in_ap=tabO[:], idxs_ap=gidxt[:, bi * 64 + 32:bi * 64 + 64],
                                    channels=128, num_elems=256, d=4, num_idxs=512)
                for k in range(4):
                    t = bi * 4 + k
                    uf = L.tile([128, 512], f32, tag="uf", bufs=6)
                    nc.sync.dma_start(uf[:], uti.ap()[t])
                    u16 = L.tile([128, 512], bf16, tag="u16", bufs=6)
                    nc.vector.tensor_copy(u16[:], uf[:])
                    mi = L.tile([128, 128, 4], bf16, tag="mi", bufs=6)
                    nc.vector.tensor_tensor(out=mi[:].rearrange("p a b -> p (a b)"),
                                            in0=g1[:, k * 128:(k + 1) * 128, :].rearrange("p a b -> p (a b)"),
                                            in1=g2[:, k * 128:(k + 1) * 128, :].rearrange("p a b -> p (a b)"),
                                            op=AL.mult)
                    pr = L.tile([128, 128, 4], bf16, tag="pr", bufs=6)
                    nc.vector.tensor_tensor(out=pr[:].rearrange("p a b -> p (a b)"),
                                            in0=mi[:].rearrange("p a b -> p (a b)"),
                                            in1=u16[:].rearrange("p e -> p e"), op=AL.mult)
                    fps = PSF.tile([128, 1], f32, space="PSUM", tag="fps", bufs=4)
                    for c in range(4):
                        nc.tensor.matmul(out=fps[:], lhsT=pr[:, :, c], rhs=ones_t[:],
                                         start=(c == 0), stop=(c == 3))
                    fb = L.tile([128, 1], f32, tag="fb", bufs=8)
                    nc.vector.tensor_scalar(out=fb[:], in0=fps[:], scalar1=wbr[:], scalar2=None, op0=AL.add)
                    ohi = L.tile([128, 256], bf16, tag="ohi", bufs=8)
                    nc.vector.tensor_scalar(out=ohi[:], in0=iota_t[:], scalar1=icolt[:, t:t + 1],
                                            scalar2=None, op0=AL.is_equal)
                    xt = L.tile([128, 256], bf16, tag="xt", bufs=8)
                    nc.vector.tensor_scalar(out=xt[:], in0=iota_t[:], scalar1=jcolt[:, t:t + 1],
                                            scalar2=fb[:], op0=AL.is_equal, op1=AL.mult)
                    for h in range(2):
                        nc.tensor.matmul(out=att_ps[h][:], lhsT=ohi[:, h * 128:(h + 1) * 128],
                                         rhs=xt[:], start=False, stop=False)

            for bi in range(NB):
                batch_body(bi)

            # close psum accumulation
            for h in range(2):
                nc.tensor.matmul(out=att_ps[h][:], lhsT=zero16[:, :128], rhs=zero16[:],
                                 start=False, stop=True)

            # ---------- epilogue ----------
            att16 = P.tile([128, 2, 256], bf16)
            attf = P.tile([128, 2, 256], f32)
            for h in range(2):
                am = P.tile([128, 256], f32, tag=f"am{h}")
                nc.vector.tensor_tensor(out=am[:], in0=att_ps[h][:], in1=eye_t[:, h, :], op=AL.subtract)
                mx = P.tile([128, 1], f32, tag=f"mx{h}")
                nc.vector.tensor_reduce(out=mx[:], in_=am[:], axis=mybir.AxisListType.X, op=AL.max)
                nmx = P.tile([128, 1], f32, tag=f"nmx{h}")
                nc.vector.tensor_scalar(out=nmx[:], in0=mx[:], scalar1=-1.0, scalar2=None, op0=AL.mult)
                ex = P.tile([128, 256], f32, tag=f"ex{h}")
                sm = P.tile([128, 1], f32, tag=f"sm{h}")
                nc.scalar.activation(out=ex[:], in_=am[:], func=AF.Exp, bias=nmx[:], scale=1.0,
                                     accum_out=sm[:])
                rs = P.tile([128, 1], f32, tag=f"rsm{h}")
                nc.vector.reciprocal(rs[:], sm[:])
                nc.scalar.activation(out=attf[:, h, :], in_=ex[:], func=AF.Copy, scale=rs[:])
                nc.vector.tensor_copy(att16[:, h, :], attf[:, h, :])
                nc.sync.dma_start(att_o.ap()[h], attf[:, h, :])
            # attT for ctx matmul
            attT = P.tile([128, 2, 256], bf16)  # [128b, bk, 256a]
            for bk in range(2):
                tp = PSW.tile([128, 256], bf16, space="PSUM", tag="w")
                for h in range(2):
                    nc.tensor.transpose(out=tp[:, h * 128:(h + 1) * 128],
                                        in_=att16[:, h, bk * 128:(bk + 1) * 128], identity=identb[:])
                nc.vector.tensor_copy(attT[:, bk, :], tp[:])
            outfeat = P.tile([128, 2, 512], f32)
            ctxf = P.tile([128, 2, 512], f32)
            for h in range(2):
                cxp = PSW.tile([128, 512], f32, space="PSUM", tag="w")
                for bk in range(2):
                    nc.tensor.matmul(out=cxp[:], lhsT=attT[:, bk, h * 128:(h + 1) * 128],
                                     rhs=c16[:, bk, :], start=(bk == 0), stop=(bk == 1))
                nc.vector.tensor_copy(ctxf[:, h, :], cxp[:])
                nc.sync.dma_start(ctx_o.ap()[h], ctxf[:, h, :])
                nc.vector.tensor_tensor(out=outfeat[:, h, :], in0=cxp[:], in1=objs[:, h, :], op=AL.add)
            # LN2 + MLP
            h16 = [P.tile([128, 512], bf16, tag=f"h16_{h}", name=f"h16_{h}") for h in range(2)]
            for h in range(2):
                layernorm(outfeat[:, h, :], ln2g, ln2b, h16[h], f"ln2{h}")
            hT = P.tile([128, 4, 256], bf16)
            for dk in range(4):
                tp = PSW.tile([128, 256], bf16, space="PSUM", tag="w")
                for h in range(2):
                    nc.tensor.transpose(out=tp[:, h * 128:(h + 1) * 128],
                                        in_=h16[h][:, dk * 128:(dk + 1) * 128], identity=identb[:])
                nc.vector.tensor_copy(hT[:, dk, :], tp[:])
            t1wT16 = P.tile([128, 4, 1024], bf16)
            nc.gpsimd.dma_start(t1wT16[:], t1wT4.ap().rearrange("k p a -> p k a"))
            t2wT16 = P.tile([128, 8, 512], bf16)
            nc.gpsimd.dma_start(t2wT16[:], t2wT8.ap().rearrange("k p a -> p k a"))
            h1r = P.tile([128, 8, 256], bf16)
            for fm in range(8):
                hp = PSW.tile([128, 256], f32, space="PSUM", tag="w")
                for dk in range(4):
                    nc.tensor.matmul(out=hp[:], lhsT=t1wT16[:, dk, fm * 128:(fm + 1) * 128],
                                     rhs=hT[:, dk, :], start=(dk == 0), stop=(dk == 3))
                nc.scalar.activation(out=h1r[:, fm, :], in_=hp[:], func=AF.Relu, bias=t1b[:, fm:fm + 1], scale=1.0)
            rt16 = P.tile([128, 4, 256], bf16)
            for dm in range(4):
                rp = PSW.tile([128, 256], f32, space="PSUM", tag="w")
                for fk in range(8):
                    nc.tensor.matmul(out=rp[:], lhsT=t2wT16[:, fk, dm * 128:(dm + 1) * 128],
                                     rhs=h1r[:, fk, :], start=(fk == 0), stop=(fk == 7))
                nc.vector.tensor_copy(rt16[:, dm, :], rp[:])
            for h in range(2):
                rn = PSW.tile([128, 512], bf16, space="PSUM", tag="w")
                for dm in range(4):
                    nc.tensor.transpose(out=rn[:, dm * 128:(dm + 1) * 128],
                                        in_=rt16[:, dm, h * 128:(h + 1) * 128], identity=identb[:])
                s1 = P.tile([128, 512], f32, tag=f"fs1{h}")
                nc.vector.tensor_tensor(out=s1[:], in0=rn[:], in1=outfeat[:, h, :], op=AL.add)
                s2 = P.tile([128, 512], f32, tag=f"fs2{h}")
                nc.vector.tensor_tensor(out=s2[:], in0=s1[:], in1=t2bb[:], op=AL.add)
                s3 = P.tile([128, 512], f32, tag=f"fs3{h}")
                nc.vector.tensor_scalar(out=s3[:], in0=s2[:], scalar1=0.0, scalar2=None, op0=AL.max)
                nc.sync.dma_start(refined_o.ap()[h], s3[:])

    nc.finalize()
    return nc


def _wrap_idx(ids):
    # ids: [512] int array -> wrapped [128, 32] int16 (idx k at [16g + k%16, k//16])
    w = ids.reshape(32, 16).T.astype(np.int16)  # [16, 32]
    return np.tile(w, (8, 1))


def _prep_core(obj, phr, pairs, params):
    (ws_w, ws_b, wo_w, wo_b, w_w, w_b, conv_w, conv_b,
     ln1_g, ln1_b, ln2_g, ln2_b, t1_w, t1_b, t2_w, t2_b) = params
    i_e = np.ascontiguousarray(pairs[:, 0]).astype(np.int64)
    j_e = np.ascontiguousarray(pairs[:, 1]).astype(np.int64)
    # uti[t, dp, (e,c)] = phr[128t+e, 128c+dp]
    phrT = np.ascontiguousarray(phr.T)                       # [512, 65536]
    uti = np.ascontiguousarray(
        phrT.reshape(4, 128, NT, 128).transpose(2, 1, 3, 0)  # [t, dp, e, c]
    ).reshape(NT, 128, 512).astype(np.float32)
    icol = i_e.reshape(NT, 128).T.astype(np.float32).copy()  # [128, NT]
    jcol = j_e.reshape(NT, 128).T.astype(np.float32).copy()
    gidx = np.zeros((128, NB * 64), np.int16)
    for b in range(NB):
        gidx[:, b * 64:b * 64 + 32] = _wrap_idx(i_e[b * 512:(b + 1) * 512])
        gidx[:, b * 64 + 32:b * 64 + 64] = _wrap_idx(j_e[b * 512:(b + 1) * 512])
    def chunks(v, n):  # bias vector [n*128] -> [128, n]
        return v.reshape(n, 128).T.astype(np.float32).copy()
    return {
        "uti": uti,
        "obj2": obj.reshape(2, 128, 512).astype(np.float32),
        "objT4": np.ascontiguousarray(obj.T).reshape(4, 128, 256).astype(np.float32),
        "wswT4": np.ascontiguousarray(ws_w.T).reshape(4, 128, 512).astype(np.float32),
        "wowT4": np.ascontiguousarray(wo_w.T).reshape(4, 128, 512).astype(np.float32),
        "convwT4": np.ascontiguousarray(conv_w.T).reshape(4, 128, 512).astype(np.float32),
        "t1wT4": np.ascontiguousarray(t1_w.T).reshape(4, 128, 1024).astype(np.float32),
        "t2wT8": np.ascontiguousarray(t2_w.T).reshape(8, 128, 512).astype(np.float32),
        "wsb4": chunks(ws_b, 4), "wob4": chunks(wo_b, 4),
        "wwT4": chunks(w_w[0], 4), "t1b8": chunks(t1_b, 8),
        "convb_b": np.tile(conv_b, (128, 1)).astype(np.float32),
        "t2b_b": np.tile(t2_b, (128, 1)).astype(np.float32),
        "ln1g_b": np.tile(ln1_g, (128, 1)).astype(np.float32),
        "ln1b_b": np.tile(ln1_b, (128, 1)).astype(np.float32),
        "ln2g_b": np.tile(ln2_g, (128, 1)).astype(np.float32),
        "ln2b_b": np.tile(ln2_b, (128, 1)).astype(np.float32),
        "wb_rep": np.full((128, 1), float(np.asarray(w_b).reshape(-1)[0]), np.float32),
        "icol": icol, "jcol": jcol, "gidx": gidx,
    }


def kernel(obj_feats, phr_feats, pair_idxs,
           ws_w, ws_b, wo_w, wo_b, w_w, w_b,
           conv_w, conv_b, ln1_g, ln1_b, ln2_g, ln2_b,
           t1_w, t1_b, t2_w, t2_b):
    global _CACHED_NC, LAST_RESULTS
    obj_feats = np.asarray(obj_feats, np.float32)
    phr_feats = np.asarray(phr_feats, np.float32)
    pairs = np.asarray(pair_idxs)
    params = tuple(np.asarray(p, np.float32) for p in
                   (ws_w, ws_b, wo_w, wo_b, w_w, w_b, conv_w, conv_b,
                    ln1_g, ln1_b, ln2_g, ln2_b, t1_w, t1_b, t2_w, t2_b))
    if _CACHED_NC is None:
        _CACHED_NC = _build_nc()
    nc = _CACHED_NC
    in_maps = [_prep_core(obj_feats[g], phr_feats[g], pairs[g], params) for g in range(B)]
    res = run_bass_kernel_spmd(nc, in_maps, core_ids=list(range(8)))
    LAST_RESULTS = res
    refined = np.stack([res.results[g]["refined_o"].reshape(256, 512) for g in range(B)])
    att = np.stack([res.results[g]["att_o"].reshape(256, 256) for g in range(B)])
    ctx = np.stack([res.results[g]["ctx_o"].reshape(256, 512) for g in range(B)])
    return refined, att, ctx


# revision 8
# speedup vs baseline: 1.3335x; 1.2268x over previous
"""Trainium2 Bass kernel for nn_ART_block (gnn_message_passing).
Data-parallel over B=8 graphs across 8 NeuronCores. Self-contained.
"""
import numpy as np
import ml_dtypes
import concourse.bass as bass
import concourse.mybir as mybir
import concourse.tile as tile
from concourse import bacc
from concourse.bass_utils import run_bass_kernel_spmd

f32 = mybir.dt.float32
bf16 = mybir.dt.bfloat16
i16 = mybir.dt.int16
AL = mybir.AluOpType
AF = mybir.ActivationFunctionType

B, N, E, D = 8, 256, 65536, 512
NT = E // 128          # 512 tiles of 128 edges
NB = NT // 4           # 128 batches of 4 tiles
EPS = 1e-5

_CACHED_NC = None
LAST_RESULTS = None


def _build_nc():
    nc = bacc.Bacc("TRN2", target_bir_lowering=False, debug=False, num_devices=8)
    # ---- inputs (per-core shard) ----
    uti = nc.dram_tensor("uti", [NT, 128, 512], f32, kind="ExternalInput")
    obj2 = nc.dram_tensor("obj2", [2, 128, 512], f32, kind="ExternalInput")
    objT4 = nc.dram_tensor("objT4", [4, 128, 256], f32, kind="ExternalInput")
    wswT4 = nc.dram_tensor("wswT4", [4, 128, 512], f32, kind="ExternalInput")
    wowT4 = nc.dram_tensor("wowT4", [4, 128, 512], f32, kind="ExternalInput")
    convwT4 = nc.dram_tensor("convwT4", [4, 128, 512], f32, kind="ExternalInput")
    t1wT4 = nc.dram_tensor("t1wT4", [4, 128, 1024], f32, kind="ExternalInput")
    t2wT8 = nc.dram_tensor("t2wT8", [8, 128, 512], f32, kind="ExternalInput")
    wsb4 = nc.dram_tensor("wsb4", [128, 4], f32, kind="ExternalInput")
    wob4 = nc.dram_tensor("wob4", [128, 4], f32, kind="ExternalInput")
    wwT4 = nc.dram_tensor("wwT4", [128, 4], f32, kind="ExternalInput")
    t1b8 = nc.dram_tensor("t1b8", [128, 8], f32, kind="ExternalInput")
    convb_b = nc.dram_tensor("convb_b", [128, 512], f32, kind="ExternalInput")
    t2b_b = nc.dram_tensor("t2b_b", [128, 512], f32, kind="ExternalInput")
    ln1g_b = nc.dram_tensor("ln1g_b", [128, 512], f32, kind="ExternalInput")
    ln1b_b = nc.dram_tensor("ln1b_b", [128, 512], f32, kind="ExternalInput")
    ln2g_b = nc.dram_tensor("ln2g_b", [128, 512], f32, kind="ExternalInput")
    ln2b_b = nc.dram_tensor("ln2b_b", [128, 512], f32, kind="ExternalInput")
    wb_rep = nc.dram_tensor("wb_rep", [128, 1], f32, kind="ExternalInput")
    icol = nc.dram_tensor("icol", [128, NT], f32, kind="ExternalInput")
    jcol = nc.dram_tensor("jcol", [128, NT], f32, kind="ExternalInput")
    gidx = nc.dram_tensor("gidx", [128, NB * 64], i16, kind="ExternalInput")
    # ---- outputs ----
    refined_o = nc.dram_tensor("refined_o", [2, 128, 512], f32, kind="ExternalOutput")
    att_o = nc.dram_tensor("att_o", [2, 128, 256], f32, kind="ExternalOutput")
    ctx_o = nc.dram_tensor("ctx_o", [2, 128, 512], f32, kind="ExternalOutput")

    # ---- inline consts ----
    iota_c = nc.inline_tensor(np.tile(np.arange(256), (128, 1)).astype(ml_dtypes.bfloat16), name="iota_c")
    ones_c = nc.inline_tensor(np.ones((128, 1), dtype=ml_dtypes.bfloat16), name="ones_c")
    identb_c = nc.inline_tensor(np.eye(128).astype(ml_dtypes.bfloat16), name="identb_c")
    identf_c = nc.inline_tensor(np.eye(128, dtype=np.float32), name="identf_c")
    eye_np = np.zeros((2, 128, 256), np.float32)
    for h in range(2):
        for p in range(128):
            eye_np[h, p, h * 128 + p] = 10000.0
    eye_c = nc.inline_tensor(eye_np, name="eye_c")

    with tile.TileContext(nc) as tc:
        with (
            tc.tile_pool(name="pers", bufs=1) as P,           # persistent SBUF
            tc.tile_pool(name="loop", bufs=3) as L,           # streaming tiles
            tc.tile_pool(name="psA", bufs=1, space="PSUM") as PSA,   # att banks
            tc.tile_pool(name="psF", bufs=2, space="PSUM") as PSF,   # atten_f
            tc.tile_pool(name="psW", bufs=2, space="PSUM") as PSW,   # prologue/epilogue work
        ):
            # ---------- preload params ----------
            def load(pool, dram, shape, dtype=f32):
                t = pool.tile(shape, dtype, name=dram.name + "_sb")
                nc.sync.dma_start(t[:], dram.ap())
                return t

            iota_t = load(P, iota_c, [128, 256], bf16)
            ones_t = load(P, ones_c, [128, 1], bf16)
            identb = load(P, identb_c, [128, 128], bf16)
            eye_t = P.tile([128, 2, 256], f32)
            nc.sync.dma_start(eye_t[:], eye_c.ap().rearrange("h p b -> p h b"))
            objs = P.tile([128, 2, 512], f32)
            nc.sync.dma_start(objs[:], obj2.ap().rearrange("h p e -> p h e"))
            objT = P.tile([128, 4, 256], f32)
            nc.sync.dma_start(objT[:], objT4.ap().rearrange("k p a -> p k a"))
            wswT = P.tile([128, 4, 512], f32)
            nc.sync.dma_start(wswT[:], wswT4.ap().rearrange("k p a -> p k a"))
            wowT = P.tile([128, 4, 512], f32)
            nc.sync.dma_start(wowT[:], wowT4.ap().rearrange("k p a -> p k a"))



            wsb = load(P, wsb4, [128, 4])
            wob = load(P, wob4, [128, 4])
            wwT = load(P, wwT4, [128, 4])
            t1b = load(P, t1b8, [128, 8])
            convb = load(P, convb_b, [128, 512])
            t2bb = load(P, t2b_b, [128, 512])
            ln1g = load(P, ln1g_b, [128, 512])
            ln1b = load(P, ln1b_b, [128, 512])
            ln2g = load(P, ln2g_b, [128, 512])
            ln2b = load(P, ln2b_b, [128, 512])
            wbr = load(P, wb_rep, [128, 1])
            icolt = load(P, icol, [128, NT])
            jcolt = load(P, jcol, [128, NT])
            gidxt = load(P, gidx, [128, NB * 64], i16)

            # ---------- prologue: gather tables s'T, oT (interleaved bf16) ----------
            tabS = P.tile([128, 256, 4], bf16)
            tabO = P.tile([128, 256, 4], bf16)
            for c in range(4):
                mm = PSW.tile([128, 256], f32, space="PSUM", tag="w")
                for k in range(4):
                    nc.tensor.matmul(out=mm[:], lhsT=wswT[:, k, c * 128:(c + 1) * 128],
                                     rhs=objT[:, k, :], start=(k == 0), stop=(k == 3))
                nc.vector.tensor_scalar(out=tabS[:, :, c], in0=mm[:],
                                        scalar1=wsb[:, c:c + 1], scalar2=wwT[:, c:c + 1],
                                        op0=AL.add, op1=AL.mult)
                mo = PSW.tile([128, 256], f32, space="PSUM", tag="w")
                for k in range(4):
                    nc.tensor.matmul(out=mo[:], lhsT=wowT[:, k, c * 128:(c + 1) * 128],
                                     rhs=objT[:, k, :], start=(k == 0), stop=(k == 3))
                nc.vector.tensor_scalar(out=tabO[:, :, c], in0=mo[:],
                                        scalar1=wob[:, c:c + 1], scalar2=None, op0=AL.add)

            # ---------- prologue: conv path c16 = relu(LN1(obj) @ conv_w.T) ----------
            def layernorm(src_h, g_t, b_t, out16_h, tag):
                # src_h: [128,512] f32 SBUF; writes bf16 normalized out
                s1 = P.tile([128, 1], f32, tag=tag + "s1")
                nc.vector.tensor_reduce(out=s1[:], in_=src_h[:], axis=mybir.AxisListType.X, op=AL.add)
                nm = P.tile([128, 1], f32, tag=tag + "nm")
                nc.vector.tensor_scalar(out=nm[:], in0=s1[:], scalar1=-1.0 / 512, scalar2=None, op0=AL.mult)
                xm = P.tile([128, 512], f32, tag="lnxm", name="lnxm")
                nc.scalar.activation(out=xm[:], in_=src_h[:], func=AF.Identity, bias=nm[:], scale=1.0)
                ssq = P.tile([128, 1], f32, tag=tag + "sq")
                junk = P.tile([128, 512], bf16, tag="lnjk", name="lnjk")
                nc.vector.scalar_tensor_tensor(out=junk[:], in0=xm[:], scalar=1.0, in1=xm[:],
                                               op0=AL.mult, op1=AL.mult, accum_out=ssq[:])
                var = P.tile([128, 1], f32, tag=tag + "vr")
                nc.vector.tensor_scalar(out=var[:], in0=ssq[:], scalar1=1.0 / 512, scalar2=EPS,
                                        op0=AL.mult, op1=AL.add)
                sd = P.tile([128, 1], f32, tag=tag + "sd")
                nc.scalar.activation(out=sd[:], in_=var[:], func=AF.Sqrt)
                rs = P.tile([128, 1], f32, tag=tag + "rs")
                nc.vector.reciprocal(rs[:], sd[:])
                xn = P.tile([128, 512], f32, tag="lnxn", name="lnxn")
                nc.scalar.activation(out=xn[:], in_=xm[:], func=AF.Copy, scale=rs[:])
                xg = P.tile([128, 512], f32, tag="lnxg", name="lnxg")
                nc.vector.tensor_tensor(out=xg[:], in0=xn[:], in1=g_t[:], op=AL.mult)
                xf = P.tile([128, 512], f32, tag="lnxf", name="lnxf")
                nc.vector.tensor_tensor(out=xf[:], in0=xg[:], in1=b_t[:], op=AL.add)
                nc.vector.tensor_copy(out16_h[:], xf[:])
                return xf

            ln1_16 = [P.tile([128, 512], bf16, tag=f"l1_{h}", name=f"l1_{h}") for h in range(2)]
            for h in range(2):
                layernorm(objs[:, h, :], ln1g, ln1b, ln1_16[h], f"ln1{h}")
            # transpose LN1 -> lnT [4][128dp, 256b] bf16
            lnT = P.tile([128, 4, 256], bf16)
            for dk in range(4):
                tp = PSW.tile([128, 256], bf16, space="PSUM", tag="w")
                for h in range(2):
                    nc.tensor.transpose(out=tp[:, h * 128:(h + 1) * 128],
                                        in_=ln1_16[h][:, dk * 128:(dk + 1) * 128], identity=identb[:])
                nc.vector.tensor_copy(lnT[:, dk, :], tp[:])
            convwT16 = P.tile([128, 4, 512], bf16)
            nc.gpsimd.dma_start(convwT16[:], convwT4.ap().rearrange("k p a -> p k a"))
            c16 = P.tile([128, 2, 512], bf16)
            for h in range(2):
                cp = PSW.tile([128, 512], f32, space="PSUM", tag="w")
                for dk in range(4):
                    nc.tensor.matmul(out=cp[:], lhsT=lnT[:, dk, h * 128:(h + 1) * 128],
                                     rhs=convwT16[:, dk, :], start=(dk == 0), stop=(dk == 3))
                cb = P.tile([128, 512], f32, tag=f"cb{h}")
                nc.vector.tensor_tensor(out=cb[:], in0=cp[:], in1=convb[:], op=AL.add)
                nc.vector.tensor_scalar(out=c16[:, h, :], in0=cb[:], scalar1=0.0, scalar2=None, op0=AL.max)

            # ---------- att PSUM init (start=True zero matmul) ----------
            att_ps = [PSA.tile([128, 256], f32, space="PSUM", tag=f"att{h}", name=f"att{h}") for h in range(2)]
            zero16 = P.tile([128, 256], bf16)
            nc.vector.memset(zero16[:], 0.0)
            for h in range(2):
                nc.tensor.matmul(out=att_ps[h][:], lhsT=zero16[:, :128], rhs=zero16[:],
                                 start=True, stop=False)

            # ---------- streaming: fully unrolled static ----------
            def batch_body(bi):
                g1 = L.tile([128, 512, 4], bf16, tag="g1", bufs=2)
                g2 = L.tile([128, 512, 4], bf16, tag="g2", bufs=2)
                nc.gpsimd.ap_gather(out_ap=g1[:], in_ap=tabS[:], idxs_ap=gidxt[:, bi * 64:bi * 64 + 32],
                                    channels=128, num_elems=256, d=4, num_idxs=512)
                nc.gpsimd.ap_gather(out_ap=g2[:], in_ap=tabO[:], idxs_ap=gidxt[:, bi * 64 + 32:bi * 64 + 64],
                                    channels=128, num_elems=256, d=4, num_idxs=512)
                for k in range(4):
                    t = bi * 4 + k
                    uf = L.tile([128, 512], f32, tag="uf", bufs=10)
                    if t % 2 == 0:
                        nc.sync.dma_start(uf[:], uti.ap()[t])
                    else:
                        nc.scalar.dma_start(uf[:], uti.ap()[t])
                    u16 = L.tile([128, 512], bf16, tag="u16", bufs=6)
                    nc.scalar.activation(out=u16[:], in_=uf[:], func=AF.Copy)
                    mi = L.tile([128, 128, 4], bf16, tag="mi", bufs=6)
                    nc.vector.tensor_tensor(out=mi[:].rearrange("p a b -> p (a b)"),
                                            in0=g1[:, k * 128:(k + 1) * 128, :].rearrange("p a b -> p (a b)"),
                                            in1=g2[:, k * 128:(k + 1) * 128, :].rearrange("p a b -> p (a b)"),
                                            op=AL.mult)
                    pr = L.tile([128, 128, 4], bf16, tag="pr", bufs=6)
                    nc.vector.tensor_tensor(out=pr[:].rearrange("p a b -> p (a b)"),
                                            in0=mi[:].rearrange("p a b -> p (a b)"),
                                            in1=u16[:].rearrange("p e -> p e"), op=AL.mult)
                    fps = PSF.tile([128, 1], f32, space="PSUM", tag="fps", bufs=4)
                    for c in range(4):
                        nc.tensor.matmul(out=fps[:], lhsT=pr[:, :, c], rhs=ones_t[:],
                                         start=(c == 0), stop=(c == 3))
                    fb = L.tile([128, 1], f32, tag="fb", bufs=8)
                    nc.scalar.activation(out=fb[:], in_=fps[:], func=AF.Identity, bias=wbr[:], scale=1.0)
                    ohi = L.tile([128, 256], bf16, tag="ohi", bufs=8)
                    nc.vector.tensor_scalar(out=ohi[:], in0=iota_t[:], scalar1=icolt[:, t:t + 1],
                                            scalar2=None, op0=AL.is_equal)
                    xt = L.tile([128, 256], bf16, tag="xt", bufs=8)
                    nc.vector.tensor_scalar(out=xt[:], in0=iota_t[:], scalar1=jcolt[:, t:t + 1],
                                            scalar2=fb[:], op0=AL.is_equal, op1=AL.mult)
                    for h in range(2):
                        nc.tensor.matmul(out=att_ps[h][:], lhsT=ohi[:, h * 128:(h + 1) * 128],
                                         rhs=xt[:], start=False, stop=False)

            for bi in range(NB):
                batch_body(bi)

            # close psum accumulation
            for h in range(2):
                nc.tensor.matmul(out=att_ps[h][:], lhsT=zero16[:, :128], rhs=zero16[:],
                                 start=False, stop=True)

            # ---------- epilogue ----------
            att16 = P.tile([128, 2, 256], bf16)
            attf = P.tile([128, 2, 256], f32)
            for h in range(2):
                am = P.tile([128, 256], f32, tag=f"am{h}")
                nc.vector.tensor_tensor(out=am[:], in0=att_ps[h][:], in1=eye_t[:, h, :], op=AL.subtract)
                mx = P.tile([128, 1], f32, tag=f"mx{h}")
                nc.vector.tensor_reduce(out=mx[:], in_=am[:], axis=mybir.AxisListType.X, op=AL.max)
                nmx = P.tile([128, 1], f32, tag=f"nmx{h}")
                nc.vector.tensor_scalar(out=nmx[:], in0=mx[:], scalar1=-1.0, scalar2=None, op0=AL.mult)
                ex = P.tile([128, 256], f32, tag=f"ex{h}")
                sm = P.tile([128, 1], f32, tag=f"sm{h}")
                nc.scalar.activation(out=ex[:], in_=am[:], func=AF.Exp, bias=nmx[:], scale=1.0,
                                     accum_out=sm[:])
                rs = P.tile([128, 1], f32, tag=f"rsm{h}")
                nc.vector.reciprocal(rs[:], sm[:])
                nc.scalar.activation(out=attf[:, h, :], in_=ex[:], func=AF.Copy, scale=rs[:])
                nc.vector.tensor_copy(att16[:, h, :], attf[:, h, :])
                nc.sync.dma_start(att_o.ap()[h], attf[:, h, :])
            # attT for ctx matmul
            attT = P.tile([128, 2, 256], bf16)  # [128b, bk, 256a]
            for bk in range(2):
                tp = PSW.tile([128, 256], bf16, space="PSUM", tag="w")
                for h in range(2):
                    nc.tensor.transpose(out=tp[:, h * 128:(h + 1) * 128],
                                        in_=att16[:, h, bk * 128:(bk + 1) * 128], identity=identb[:])
                nc.vector.tensor_copy(attT[:, bk, :], tp[:])
            outfeat = P.tile([128, 2, 512], f32)
            ctxf = P.tile([128, 2, 512], f32)
            for h in range(2):
                cxp = PSW.tile([128, 512], f32, space="PSUM", tag="w")
                for bk in range(2):
                    nc.tensor.matmul(out=cxp[:], lhsT=attT[:, bk, h * 128:(h + 1) * 128],
                                     rhs=c16[:, bk, :], start=(bk == 0), stop=(bk == 1))
                nc.vector.tensor_copy(ctxf[:, h, :], cxp[:])
                nc.sync.dma_start(ctx_o.ap()[h], ctxf[:, h, :])
                nc.vector.tensor_tensor(out=outfeat[:, h, :], in0=cxp[:], in1=objs[:, h, :], op=AL.add)
            # LN2 + MLP
            h16 = [P.tile([128, 512], bf16, tag=f"h16_{h}", name=f"h16_{h}") for h in range(2)]
            for h in range(2):
                layernorm(outfeat[:, h, :], ln2g, ln2b, h16[h], f"ln2{h}")
            hT = P.tile([128, 4, 256], bf16)
            for dk in range(4):
                tp = PSW.tile([128, 256], bf16, space="PSUM", tag="w")
                for h in range(2):
                    nc.tensor.transpose(out=tp[:, h * 128:(h + 1) * 128],
                                        in_=h16[h][:, dk * 128:(dk + 1) * 128], identity=identb[:])
                nc.vector.tensor_copy(hT[:, dk, :], tp[:])
            t1wT16 = P.tile([128, 4, 1024], bf16)
            nc.gpsimd.dma_start(t1wT16[:], t1wT4.ap().rearrange("k p a -> p k a"))
            t2wT16 = P.tile([128, 8, 512], bf16)
            nc.gpsimd.dma_start(t2wT16[:], t2wT8.ap().rearrange("k p a -> p k a"))
            h1r = P.tile([128, 8, 256], bf16)
            for fm in range(8):
                hp = PSW.tile([128, 256], f32, space="PSUM", tag="w")
                for dk in range(4):
                    nc.tensor.matmul(out=hp[:], lhsT=t1wT16[:, dk, fm * 128:(fm + 1) * 128],
                                     rhs=hT[:, dk, :], start=(dk == 0), stop=(dk == 3))
                nc.scalar.activation(out=h1r[:, fm, :], in_=hp[:], func=AF.Relu, bias=t1b[:, fm:fm + 1], scale=1.0)
            rt16 = P.tile([128, 4, 256], bf16)
            for dm in range(4):
                rp = PSW.tile([128, 256], f32, space="PSUM", tag="w")
                for fk in range(8):
                    nc.tensor.matmul(out=rp[:], lhsT=t2wT16[:, fk, dm * 128:(dm + 1) * 128],
                                     rhs=h1r[:, fk, :], start=(fk == 0), stop=(fk == 7))
                nc.vector.tensor_copy(rt16[:, dm, :], rp[:])
            for h in range(2):
                rn = PSW.tile([128, 512], bf16, space="PSUM", tag="w")
                for dm in range(4):
                    nc.tensor.transpose(out=rn[:, dm * 128:(dm + 1) * 128],
                                        in_=rt16[:, dm, h * 128:(h + 1) * 128], identity=identb[:])
                s1 = P.tile([128, 512], f32, tag=f"fs1{h}")
                nc.vector.tensor_tensor(out=s1[:], in0=rn[:], in1=outfeat[:, h, :], op=AL.add)
                s2 = P.tile([128, 512], f32, tag=f"fs2{h}")
                nc.vector.tensor_tensor(out=s2[:], in0=s1[:], in1=t2bb[:], op=AL.add)
                s3 = P.tile([128, 512], f32, tag=f"fs3{h}")
                nc.vector.tensor_scalar(out=s3[:], in0=s2[:], scalar1=0.0, scalar2=None, op0=AL.max)
                nc.sync.dma_start(refined_o.ap()[h], s3[:])

    nc.finalize()
    return nc


def _wrap_idx(ids):
    # ids: [512] int array -> wrapped [128, 32] int16 (idx k at [16g + k%16, k//16])
    w = ids.reshape(32, 16).T.astype(np.int16)  # [16, 32]
    return np.tile(w, (8, 1))


def _prep_core(obj, phr, pairs, params):
    (ws_w, ws_b, wo_w, wo_b, w_w, w_b, conv_w, conv_b,
     ln1_g, ln1_b, ln2_g, ln2_b, t1_w, t1_b, t2_w, t2_b) = params
    i_e = np.ascontiguousarray(pairs[:, 0]).astype(np.int64)
    j_e = np.ascontiguousarray(pairs[:, 1]).astype(np.int64)
    # uti[t, dp, (e,c)] = phr[128t+e, 128c+dp]
    phrT = np.ascontiguousarray(phr.T)                       # [512, 65536]
    uti = np.ascontiguousarray(
        phrT.reshape(4, 128, NT, 128).transpose(2, 1, 3, 0)  # [t, dp, e, c]
    ).reshape(NT, 128, 512).astype(np.float32)
    icol = i_e.reshape(NT, 128).T.astype(np.float32).copy()  # [128, NT]
    jcol = j_e.reshape(NT, 128).T.astype(np.float32).copy()
    gidx = np.zeros((128, NB * 64), np.int16)
    for b in range(NB):
        gidx[:, b * 64:b * 64 + 32] = _wrap_idx(i_e[b * 512:(b + 1) * 512])
        gidx[:, b * 64 + 32:b * 64 + 64] = _wrap_idx(j_e[b * 512:(b + 1) * 512])
    def chunks(v, n):  # bias vector [n*128] -> [128, n]
        return v.reshape(n, 128).T.astype(np.float32).copy()
    return {
        "uti": uti,
        "obj2": obj.reshape(2, 128, 512).astype(np.float32),
        "objT4": np.ascontiguousarray(obj.T).reshape(4, 128, 256).astype(np.float32),
        "wswT4": np.ascontiguousarray(ws_w.T).reshape(4, 128, 512).astype(np.float32),
        "wowT4": np.ascontiguousarray(wo_w.T).reshape(4, 128, 512).astype(np.float32),
        "convwT4": np.ascontiguousarray(conv_w.T).reshape(4, 128, 512).astype(np.float32),
        "t1wT4": np.ascontiguousarray(t1_w.T).reshape(4, 128, 1024).astype(np.float32),
        "t2wT8": np.ascontiguousarray(t2_w.T).reshape(8, 128, 512).astype(np.float32),
        "wsb4": chunks(ws_b, 4), "wob4": chunks(wo_b, 4),
        "wwT4": chunks(w_w[0], 4), "t1b8": chunks(t1_b, 8),
        "convb_b": np.tile(conv_b, (128, 1)).astype(np.float32),
        "t2b_b": np.tile(t2_b, (128, 1)).astype(np.float32),
        "ln1g_b": np.tile(ln1_g, (128, 1)).astype(np.float32),
        "ln1b_b": np.tile(ln1_b, (128, 1)).astype(np.float32),
        "ln2g_b": np.tile(ln2_g, (128, 1)).astype(np.float32),
        "ln2b_b": np.tile(ln2_b, (128, 1)).astype(np.float32),
        "wb_rep": np.full((128, 1), float(np.asarray(w_b).reshape(-1)[0]), np.float32),
        "icol": icol, "jcol": jcol, "gidx": gidx,
    }


def kernel(obj_feats, phr_feats, pair_idxs,
           ws_w, ws_b, wo_w, wo_b, w_w, w_b,
           conv_w, conv_b, ln1_g, ln1_b, ln2_g, ln2_b,
           t1_w, t1_b, t2_w, t2_b):
    global _CACHED_NC, LAST_RESULTS
    obj_feats = np.asarray(obj_feats, np.float32)
    phr_feats = np.asarray(phr_feats, np.float32)
    pairs = np.asarray(pair_idxs)
    params = tuple(np.asarray(p, np.float32) for p in
                   (ws_w, ws_b, wo_w, wo_b, w_w, w_b, conv_w, conv_b,
                    ln1_g, ln1_b, ln2_g, ln2_b, t1_w, t1_b, t2_w, t2_b))
    if _CACHED_NC is None:
        _CACHED_NC = _build_nc()
    nc = _CACHED_NC
    in_maps = [_prep_core(obj_feats[g], phr_feats[g], pairs[g], params) for g in range(B)]
    res = run_bass_kernel_spmd(nc, in_maps, core_ids=list(range(8)))
    LAST_RESULTS = res
    refined = np.stack([res.results[g]["refined_o"].reshape(256, 512) for g in range(B)])
    att = np.stack([res.results[g]["att_o"].reshape(256, 256) for g in range(B)])
    ctx = np.stack([res.results[g]["ctx_o"].reshape(256, 512) for g in range(B)])
    return refined, att, ctx
